# revision 35
# baseline (speedup 1.0000x reference)
"""nn_DiTBlock on 8 TRN2 NeuronCores: data-parallel over batch (B=8), one
batch element per core. Self-contained: builds the Bass/Tile kernel, shards
inputs on the host (transpose/pack/cast only), runs SPMD via bass2jax/PJRT,
gathers and un-transposes the output.

v2 design: fp8e4(e4m3)+DoubleRow matmuls for qkv/v/attn@v/proj/fc1/fc2
(weights host-prescaled x128, descale folded into evictions), bf16 for adaLN
and attention scores, f32 residual + PSUM. exp scaled by 1/32 (cancels in
softmax); k-rmsnorm folded into the exp's per-partition scale; odd heads'
attn@v written directly to PSUM partitions 64:128 (no partition-move DMAs);
single-pass MLP; multi-chunk batched weight DMAs."""

import numpy as np
from contextlib import ExitStack

import concourse.bass as bass
import concourse.mybir as mybir
import concourse.tile as tile
from concourse import bacc


F32 = mybir.dt.float32
F32R = mybir.dt.float32r
BF16 = mybir.dt.bfloat16
FP8 = mybir.dt.float8e4
AF = mybir.ActivationFunctionType
OP = mybir.AluOpType
DR = mybir.MatmulPerfMode.DoubleRow

B, T, D, H = 8, 1024, 1024, 16
HD = D // H          # 64
DM = 4 * D           # 4096
NCH = D // 128       # 8
MCH = DM // 128      # 32
P = 128
WS = 128.0           # fp8 weight pre-scale (host)
ISV = 1.0 / WS
ELN32 = -3.4657359027997265  # -ln(32): exp pre-scale so fp8 es stays < 240


def host_prep(x, c, g1, g2, gq, gk, Wqkv, bqkv, Wproj, bproj,
              Wfc1, bfc1, Wfc2, bfc2, Wada, bada):
    import ml_dtypes
    E4 = mybir.dt.np(FP8)

    def packT(W, npdt, scale=1.0):  # (F, K) -> (K//128, 128, F) contiguous
        Wt = np.ascontiguousarray(np.asarray(W, np.float32).T * scale).astype(npdt)
        K, F = Wt.shape
        return np.ascontiguousarray(Wt.reshape(K // 128, 128, F))

    f32 = np.float32
    com = {
        "wqkv": packT(Wqkv, E4, WS), "wproj": packT(Wproj, E4, WS),
        "wfc1": packT(Wfc1, E4, WS), "wfc2": packT(Wfc2, E4, WS),
        "wada": packT(Wada, ml_dtypes.bfloat16),
        "bqkv": np.asarray(bqkv, f32), "bproj": np.asarray(bproj, f32),
        "bfc1": np.asarray(bfc1, f32), "bfc2": np.asarray(bfc2, f32),
        "bada": np.asarray(bada, f32),
        "g": np.stack([np.asarray(g1)[0], np.asarray(g2)[0],
                       np.asarray(gq)[0], np.asarray(gk)[0]]).astype(f32),
    }
    in_maps = []
    for b in range(B):
        m = dict(com)
        m["xt"] = np.ascontiguousarray(np.asarray(x[b], f32).T)
        m["cvec"] = np.asarray(c[b], f32)
        in_maps.append(m)
    return in_maps


def host_post(results):
    return np.ascontiguousarray(
        np.stack([r["out"].T for r in results]).astype(np.float32))


def col_ap(handle, nch):
    """DRAM (nch*128,) viewed as [128, nch]: tile[p, ch] = v[ch*128+p]."""
    return bass.AP(tensor=handle, offset=0, ap=[[1, P], [P, nch]])


def bc_ap(handle, n, offset=0):
    """DRAM (n,) broadcast-read to [128, n] (partition stride 0)."""
    return bass.AP(tensor=handle, offset=offset, ap=[[0, P], [1, n]])


def wload_ap(handle, kch, cols, col0):
    """DRAM weight pack [KCH,128,F] -> [128, kch, cols] AP at col offset."""
    F = handle.shape[2]
    return bass.AP(tensor=handle, offset=col0,
                   ap=[[F, P], [P * F, kch], [1, cols]])


def build_dit(n_cores=8):
    nc = bacc.Bacc("TRN2", target_bir_lowering=False, debug=False,
                   num_devices=n_cores)

    xt = nc.dram_tensor("xt", [D, T], F32, kind="ExternalInput")
    cin = nc.dram_tensor("cvec", [D], F32, kind="ExternalInput")
    g = nc.dram_tensor("g", [4], F32, kind="ExternalInput")
    wqkv = nc.dram_tensor("wqkv", [NCH, P, 3 * D], FP8, kind="ExternalInput")
    wproj = nc.dram_tensor("wproj", [NCH, P, D], FP8, kind="ExternalInput")
    wfc1 = nc.dram_tensor("wfc1", [NCH, P, DM], FP8, kind="ExternalInput")
    wfc2 = nc.dram_tensor("wfc2", [MCH, P, D], FP8, kind="ExternalInput")
    wada = nc.dram_tensor("wada", [NCH, P, 6 * D], BF16, kind="ExternalInput")
    bqkv = nc.dram_tensor("bqkv", [3 * D], F32, kind="ExternalInput")
    bproj = nc.dram_tensor("bproj", [D], F32, kind="ExternalInput")
    bfc1 = nc.dram_tensor("bfc1", [DM], F32, kind="ExternalInput")
    bfc2 = nc.dram_tensor("bfc2", [D], F32, kind="ExternalInput")
    bada = nc.dram_tensor("bada", [6 * D], F32, kind="ExternalInput")
    out = nc.dram_tensor("out", [D, T], F32, kind="ExternalOutput")

    with tile.TileContext(nc, pool_alloc_mode="queue") as tc:
        with ExitStack() as X:
            const = X.enter_context(tc.tile_pool(name="const", bufs=1))
            resid = X.enter_context(tc.tile_pool(name="resid", bufs=1))
            dram = X.enter_context(tc.tile_pool(name="dram", bufs=1, space="DRAM"))

            # ---------------- constants ----------------
            g_bc = const.tile([P, 4], F32)
            nc.sync.dma_start(out=g_bc, in_=bc_ap(g, 4))
            gsq = const.tile([P, 4], F32)
            nc.vector.tensor_tensor(gsq, g_bc, g_bc, OP.mult)
            ginv2 = const.tile([P, 4], F32)
            nc.vector.reciprocal(ginv2, gsq)
            # Rsqrt scales: rinv = rsqrt(ps * scl)
            scl_n1 = const.tile([P, 1], F32)
            nc.vector.tensor_scalar_mul(scl_n1, ginv2[:, 0:1], 1.0 / D)
            scl_n2 = const.tile([P, 1], F32)
            nc.vector.tensor_scalar_mul(scl_n2, ginv2[:, 1:2], 1.0 / D)
            scl_q = const.tile([P, 1], F32)
            nc.vector.tensor_copy(scl_q, ginv2[:, 2:3])
            scl_k = const.tile([P, 1], F32)
            nc.vector.tensor_scalar_mul(scl_k, ginv2[:, 3:4], 1.0 / HD)

            ones1_f = const.tile([P, 1], F32)
            nc.gpsimd.memset(ones1_f, 1.0)
            ones1 = const.tile([P, 1], BF16)
            nc.vector.tensor_copy(ones1, ones1_f)
            onesh_f = const.tile([P, 2], F32)
            nc.gpsimd.memset(onesh_f, 0.0)
            nc.gpsimd.memset(onesh_f[0:64, 0:1], 1.0)
            nc.gpsimd.memset(onesh_f[64:128, 1:2], 1.0)
            onesh = const.tile([P, 2], BF16)
            nc.vector.tensor_copy(onesh, onesh_f)

            bqkv_c = const.tile([P, 3 * D // P], F32)
            nc.sync.dma_start(out=bqkv_c, in_=col_ap(bqkv, 3 * D // P))
            bproj_c = const.tile([P, NCH], F32)
            nc.sync.dma_start(out=bproj_c, in_=col_ap(bproj, NCH))
            bfc1_c = const.tile([P, MCH], F32)
            nc.sync.dma_start(out=bfc1_c, in_=col_ap(bfc1, MCH))
            bfc2_c = const.tile([P, NCH], F32)
            nc.sync.dma_start(out=bfc2_c, in_=col_ap(bfc2, NCH))
            vbias_bc = const.tile([P, D], F32)
            nc.sync.dma_start(out=vbias_bc, in_=bc_ap(bqkv, D, offset=2 * D))
            eln32_c = const.tile([P, 1], F32)
            nc.gpsimd.memset(eln32_c, ELN32)

            x_res = resid.tile([P, NCH, T], F32)
            for j in range(4):
                nc.sync.dma_start(
                    out=x_res[:, 2 * j:2 * j + 2, :],
                    in_=bass.AP(tensor=xt, offset=2 * j * P * T,
                                ap=[[T, P], [P * T, 2], [1, T]]))

            c_pm = const.tile([P, NCH], F32)
            nc.sync.dma_start(out=c_pm, in_=col_ap(cin, NCH))
            cs_pm = const.tile([P, NCH], BF16)
            nc.scalar.activation(cs_pm, c_pm, AF.Silu)

            # ---------------- adaLN (bf16) ----------------
            ada_scr = dram.tile([1, 6 * D], F32)
            ada_sb = const.tile([1, 6 * D], F32)
            with tc.tile_pool(name="wadap", bufs=3) as wp, \
                 tc.tile_pool(name="psA", bufs=2, space="PSUM") as psA:
                for nb in range(12):
                    wt = wp.tile([P, NCH, 512], BF16, name="wt")
                    nc.sync.dma_start(out=wt, in_=wload_ap(wada, NCH, 512, nb * 512))
                    pa = psA.tile([1, 512], F32, name="pa")
                    for d in range(NCH):
                        nc.tensor.matmul(pa, cs_pm[:, d:d + 1], wt[:, d, :],
                                         start=(d == 0), stop=(d == NCH - 1))
                    nc.vector.tensor_copy(ada_sb[:, nb * 512:(nb + 1) * 512], pa)
            nc.sync.dma_start(out=ada_scr, in_=ada_sb)
            adaT = const.tile([P, 48], F32)
            nc.sync.dma_start(out=adaT, in_=bass.AP(tensor=ada_scr.tensor, offset=0,
                                                    ap=[[1, P], [P, 48]]))
            badaT = const.tile([P, 48], F32)
            nc.sync.dma_start(out=badaT, in_=col_ap(bada, 48))
            nc.vector.tensor_tensor(adaT, adaT, badaT, OP.add)
            # cols: shift_msa 0:8 | scale_msa 8:16 | gate_msa 16:24
            #       shift_mlp 24:32 | scale_mlp 32:40 | gate_mlp 40:48
            nc.vector.tensor_scalar_add(adaT[:, 8:16], adaT[:, 8:16], 1.0)
            nc.vector.tensor_scalar_add(adaT[:, 32:40], adaT[:, 32:40], 1.0)
            gb_proj = const.tile([P, NCH], F32)
            nc.vector.tensor_tensor(gb_proj, adaT[:, 16:24], bproj_c, OP.mult)
            gbs_proj = const.tile([P, NCH], F32)
            nc.vector.tensor_scalar_mul(gbs_proj, adaT[:, 16:24], ISV)
            gb_fc2 = const.tile([P, NCH], F32)
            nc.vector.tensor_tensor(gb_fc2, adaT[:, 40:48], bfc2_c, OP.mult)
            gbs_fc2 = const.tile([P, NCH], F32)
            nc.vector.tensor_scalar_mul(gbs_fc2, adaT[:, 40:48], ISV)

            def norm_modulate(scl, sh_col, sc_col, h_out):
                """x_res (f32) -> h_out (fp8): rmsnorm + adaLN modulate.
                Token-halved so the consumer can start on half 0 while the
                producer of x_res is still finishing half 1."""
                with tc.tile_pool(name="sqp", bufs=3) as sqp, \
                     tc.tile_pool(name="psN", bufs=1, space="PSUM") as psN, \
                     tc.tile_pool(name="nrm", bufs=2) as nrm, \
                     tc.tile_pool(name="xnp", bufs=3) as xnp:
                    pss = psN.tile([1, T], F32, name="pss")
                    for t2 in range(2):
                        ts_ = slice(t2 * 512, (t2 + 1) * 512)
                        for j in range(NCH):
                            xsq = sqp.tile([P, 512], BF16, name="xsq")
                            nc.scalar.activation(xsq, x_res[:, j, ts_], AF.Square)
                            nc.tensor.matmul(pss[:, ts_], ones1, xsq,
                                             start=(j == 0), stop=(j == NCH - 1))
                        rr = nrm.tile([1, 512], F32, name="rr")
                        nc.scalar.activation(rr, pss[:, ts_], AF.Sqrt,
                                             scale=scl[0:1, :])
                        rinv = nrm.tile([1, 512], F32, name="rinv")
                        nc.vector.reciprocal(rinv, rr)
                        rbc = nrm.tile([P, 512], F32, name="rbc")
                        nc.gpsimd.partition_broadcast(rbc, rinv)
                        for j in range(NCH):
                            xn = xnp.tile([P, 512], F32, name="xn")
                            nc.vector.tensor_tensor(xn, x_res[:, j, ts_], rbc,
                                                    OP.mult)
                            nc.gpsimd.tensor_scalar(h_out[:, j, ts_], xn,
                                                    adaT[:, sc_col + j:sc_col + j + 1],
                                                    adaT[:, sh_col + j:sh_col + j + 1],
                                                    OP.mult, OP.add)

            # fc1 weights tile created before the attention pools (so they
            # can close first); its load is emitted at proj time, landing
            # during attention when the wire is idle
            mlpw = X.enter_context(tc.tile_pool(name="mlpw", bufs=1))
            w1a = mlpw.tile([P, NCH, DM // 2], FP8)

            att = ExitStack()
            h1p = att.enter_context(tc.tile_pool(name="h1p", bufs=1, side="right"))
            h1 = h1p.tile([P, NCH, T], FP8)
            # ------------ norm1 + modulate ------------
            norm_modulate(scl_n1, 0, 8, h1)

            # ------------ q, k (feature-major bf16) + per-head rmsnorm ------------
            qp_ = att.enter_context(tc.tile_pool(name="qp_", bufs=1))
            kp_ = att.enter_context(tc.tile_pool(name="kp_", bufs=1))
            q_t = qp_.tile([P, NCH, T], BF16)
            k_t = kp_.tile([P, NCH, T], BF16)
            rkcp = att.enter_context(tc.tile_pool(name="rkcp", bufs=1))
            rkc = rkcp.tile([P, H, NCH], F32)  # 1/|k| per k-token, head-major

            with tc.tile_pool(name="wqp", bufs=2) as wqp, \
                 tc.tile_pool(name="sqq", bufs=2) as sqq, \
                 tc.tile_pool(name="psD", bufs=2, space="PSUM") as psD, \
                 tc.tile_pool(name="psR", bufs=1, space="PSUM") as psR, \
                 tc.tile_pool(name="nrq", bufs=2) as nrq:
                for fc in range(16):  # q: 0..7, k: 8..15
                    if fc % 4 == 0:
                        wt = wqp.tile([P, NCH, 512], FP8, name="wt")
                        nc.sync.dma_start(out=wt, in_=wload_ap(wqkv, NCH, 512, fc * P))
                    tgt = q_t if fc < 8 else k_t
                    ch = fc % 8
                    ps = [psD.tile([P, 512], F32, name="ps") for _ in range(2)]
                    for nt in range(2):
                        for dp in range(4):
                            nc.tensor.matmul(
                                ps[nt],
                                wt[:, 2 * dp:2 * dp + 2, (fc % 4) * P:(fc % 4 + 1) * P],
                                h1[:, 2 * dp:2 * dp + 2, nt * 512:(nt + 1) * 512],
                                start=(dp == 0), stop=(dp == 3), perf_mode=DR)
                        # evict: (psum/128 + bias) -> bf16 on Act (idle here)
                        nc.scalar.activation(tgt[:, ch, nt * 512:(nt + 1) * 512],
                                             ps[nt], AF.Identity, scale=ISV,
                                             bias=bqkv_c[:, fc:fc + 1])
                    # sum of squares per head
                    sq = sqq.tile([P, T], BF16, name="sq")
                    nc.vector.tensor_tensor(sq, tgt[:, ch, :], tgt[:, ch, :], OP.mult)
                    if fc < 8:
                        # q: per-half sums, each in its own row-0 psum tile;
                        # broadcasts always source partition 0 into full tiles
                        for hfq in range(2):
                            prh = psR.tile([1, T], F32, name=f"prh{hfq}")
                            for nt in range(2):
                                nc.tensor.matmul(
                                    prh[:, nt * 512:(nt + 1) * 512],
                                    onesh[:, hfq:hfq + 1],
                                    sq[:, nt * 512:(nt + 1) * 512],
                                    start=True, stop=True)
                            rr2 = nrq.tile([1, T], BF16, name=f"rr2{hfq}")
                            nc.scalar.activation(rr2, prh, AF.Sqrt,
                                                 scale=scl_q[0:1, :])
                            rinv_sb = nrq.tile([1, T], BF16, name=f"ri{hfq}")
                            with nc.allow_low_precision(reason="1/|q| bf16"):
                                nc.vector.reciprocal(rinv_sb, rr2)
                            rbcq = nrq.tile([P, T], BF16, name=f"rbcq{hfq}")
                            nc.gpsimd.partition_broadcast(rbcq, rinv_sb)
                            hs = slice(64 * hfq, 64 * (hfq + 1))
                            nc.vector.tensor_tensor(q_t[hs, ch, :], q_t[hs, ch, :],
                                                    rbcq[hs, :], OP.mult)
                    else:
                        # k: sums token-major [128, NCH] per head -> 1/|k| into
                        # rkc, consumed as the exp's per-partition scale.
                        for j in range(2):
                            hidx = 2 * (fc - 8) + j
                            pkn = psR.tile([P, NCH], F32, name="pkn")
                            for kt in range(NCH):
                                nc.tensor.matmul(
                                    pkn[:, kt:kt + 1],
                                    sq[64 * j:64 * (j + 1), kt * P:(kt + 1) * P],
                                    ones1[64 * j:64 * (j + 1), :],
                                    start=True, stop=True)
                            rrk = nrq.tile([P, NCH], F32, name="rrk")
                            nc.scalar.activation(rrk, pkn, AF.Sqrt, scale=scl_k)
                            nc.vector.reciprocal(rkc[:, hidx, :], rrk)

            # ------------ v (token-major fp8, ones-augmented) ------------
            # vx per-head 128-col slot: even h = [v(0:64) | ones@64 | 0],
            # odd h = [0 | ones@63 | v(64:128)]; attn@v DR outputs are then
            # always full [128, N] (walrus requires that) and odd heads land
            # on PSUM partitions 64:128 directly.
            vxp = att.enter_context(tc.tile_pool(name="vxp", bufs=1))
            vx = vxp.tile([P, NCH, H, P], FP8)   # [ktok][ktc][head][col]
            nc.gpsimd.memset(vx, 0.0)
            for h in range(H):
                oc = HD if h % 2 == 0 else 0
                nc.gpsimd.memset(vx[:, :, h, oc:oc + 1], 1.0)
            with tc.tile_pool(name="wvp", bufs=2) as wvp, \
                 tc.tile_pool(name="psV", bufs=3, space="PSUM") as psV:
                for nq in range(2):
                    wv = wvp.tile([P, NCH, 512], FP8, name="wv")
                    nc.sync.dma_start(out=wv,
                                      in_=wload_ap(wqkv, NCH, 512, 2 * D + nq * 512))
                    for t8 in range(NCH):
                        pv = psV.tile([P, 512], F32, name="pv")
                        for dp in range(4):
                            nc.tensor.matmul(
                                pv, h1[:, 2 * dp:2 * dp + 2, t8 * P:(t8 + 1) * P],
                                wv[:, 2 * dp:2 * dp + 2, :],
                                start=(dp == 0), stop=(dp == 3), perf_mode=DR)
                        # heads alternate col-base 0 (even) / 64 (odd) in vx
                        vblk = vx[:, t8, :, :].rearrange(
                            "p h c -> p (h c)").rearrange(
                            "p (i r) -> p i r", r=256)  # [P, 8, 256]
                        for par in range(2):
                            nc.vector.scalar_tensor_tensor(
                                vblk[:, 4 * nq:4 * nq + 4,
                                     192 * par:192 * par + HD],
                                pv.rearrange("p (i r) -> p i r", r=128)[
                                    :, :, par * HD:(par + 1) * HD], ISV,
                                vbias_bc[:, nq * 512:(nq + 1) * 512].rearrange(
                                    "p (i r) -> p i r", r=128)[
                                    :, :, par * HD:(par + 1) * HD],
                                OP.mult, OP.add)

            # ------------ attention ------------
            oTp = att.enter_context(tc.tile_pool(name="oTp", bufs=1, side="right"))
            oT = oTp.tile([P, NCH, T], FP8)
            with tc.tile_pool(name="esp", bufs=2) as esp, \
                 tc.tile_pool(name="psS", bufs=2, space="PSUM") as psS, \
                 tc.tile_pool(name="psO", bufs=2, space="PSUM") as psO, \
                 tc.tile_pool(name="onp", bufs=4) as onp:
                for h in range(H):
                    hc, hf = h // 2, h % 2
                    rq = slice(64 * hf, 64 * (hf + 1))
                    es_h = esp.tile([P, NCH, T], FP8, name="es")
                    for ktc in range(NCH):
                        psc = psS.tile([P, T], F32, name="psc")
                        for qt in range(2):
                            nc.tensor.matmul(psc[:, qt * 512:(qt + 1) * 512],
                                             k_t[rq, hc, ktc * P:(ktc + 1) * P],
                                             q_t[rq, hc, qt * 512:(qt + 1) * 512],
                                             start=True, stop=True)
                        nc.scalar.activation(es_h[:, ktc, :], psc, AF.Exp,
                                             bias=eln32_c, scale=rkc[:, h, ktc:ktc + 1])
                    for qt in range(2):
                        qs = slice(qt * 512, (qt + 1) * 512)
                        po = psO.tile([P, 512], F32, name="po")
                        rs = onp.tile([P, 512], F32, name="rs")
                        rsb = onp.tile([P, 512], F32, name="rsb")
                        for kp in range(4):
                            nc.tensor.matmul(
                                po, vx[:, 2 * kp:2 * kp + 2, h, :],
                                es_h[:, 2 * kp:2 * kp + 2, qs],
                                start=(kp == 0), stop=(kp == 3), perf_mode=DR)
                        if hf == 0:
                            # denom at row 64: recip there, DMA row to
                            # partition 0, broadcast full, use rows 0:64
                            nc.vector.reciprocal(rs[64:65, :], po[64:65, :])
                            rse = onp.tile([1, 512], F32, name="rse")
                            nc.sync.dma_start(out=rse, in_=rs[64:65, :])
                            nc.gpsimd.partition_broadcast(rsb, rse)
                            nc.vector.tensor_tensor(oT[0:64, hc, qs], po[0:64, :],
                                                    rsb[0:64, :], OP.mult)
                        else:
                            # denom at row 0: broadcast full, use rows 64:128
                            nc.vector.reciprocal(rs[0:1, :], po[0:1, :])
                            nc.gpsimd.partition_broadcast(rsb, rs[0:1, :])
                            nc.vector.tensor_tensor(oT[64:128, hc, qs], po[64:128, :],
                                                    rsb[64:128, :], OP.mult)

            # ------------ proj + residual ------------
            with tc.tile_pool(name="wpp", bufs=1) as wpp, \
                 tc.tile_pool(name="psP", bufs=3, space="PSUM") as psP:
                wpj = wpp.tile([P, NCH, D], FP8, name="wpj")
                nc.sync.dma_start(out=wpj, in_=wload_ap(wproj, NCH, D, 0))
                nc.sync.dma_start(out=w1a, in_=wload_ap(wfc1, NCH, DM // 2, 0))
                for nt in range(2):
                    for fc in range(8):
                        pp = psP.tile([P, 512], F32, name="pp")
                        for dp in range(4):
                            nc.tensor.matmul(
                                pp, wpj[:, 2 * dp:2 * dp + 2, fc * P:(fc + 1) * P],
                                oT[:, 2 * dp:2 * dp + 2, nt * 512:(nt + 1) * 512],
                                start=(dp == 0), stop=(dp == 3), perf_mode=DR)
                        nc.vector.affine_then_add(
                            x_res[:, fc, nt * 512:(nt + 1) * 512], pp,
                            x_res[:, fc, nt * 512:(nt + 1) * 512],
                            scale=gbs_proj[:, fc:fc + 1],
                            bias=gb_proj[:, fc:fc + 1])

            att.close()  # free h1, q/k, vx, oT, rkc

            # ------------ norm2 + modulate + MLP (single pass, fp8) ------------
            with tc.tile_pool(name="h2p", bufs=1) as h2p, \
                 tc.tile_pool(name="gactp", bufs=1, side="right") as gactp:
                h2 = h2p.tile([P, NCH, T], FP8)
                norm_modulate(scl_n2, 24, 32, h2)
                gact = gactp.tile([P, MCH, T], FP8)
                w1b = gactp.tile([P, NCH, DM // 2], FP8, name="w1b")
                nc.sync.dma_start(out=w1b, in_=wload_ap(wfc1, NCH, DM // 2, DM // 2))
                w2 = gactp.tile([P, MCH, D], FP8, name="w2full")
                nc.sync.dma_start(out=w2, in_=wload_ap(wfc2, MCH, D, 0))
                # nt-outer: fc2 on token-half 0 overlaps fc1/gelu on half 1
                with tc.tile_pool(name="psM", bufs=3, space="PSUM") as psM, \
                     tc.tile_pool(name="psM2", bufs=3, space="PSUM") as psM2:
                    for nt in range(2):
                        ns_ = slice(nt * 512, (nt + 1) * 512)
                        for m in range(MCH):
                            psm = psM.tile([P, 512], F32, name="psm")
                            w1h = w1a if m < 16 else w1b
                            mo = m if m < 16 else m - 16
                            for dp in range(4):
                                nc.tensor.matmul(
                                    psm,
                                    w1h[:, 2 * dp:2 * dp + 2, mo * P:(mo + 1) * P],
                                    h2[:, 2 * dp:2 * dp + 2, ns_],
                                    start=(dp == 0), stop=(dp == 3), perf_mode=DR)
                            nc.scalar.activation(gact[:, m, ns_], psm,
                                                 AF.Gelu_apprx_tanh, scale=ISV,
                                                 bias=bfc1_c[:, m:m + 1])
                        for fc in range(8):
                            ps2 = psM2.tile([P, 512], F32, name="ps2")
                            for dp in range(16):
                                nc.tensor.matmul(
                                    ps2,
                                    w2[:, 2 * dp:2 * dp + 2, fc * P:(fc + 1) * P],
                                    gact[:, 2 * dp:2 * dp + 2, ns_],
                                    start=(dp == 0), stop=(dp == 15), perf_mode=DR)
                            nc.vector.affine_then_add(
                                x_res[:, fc, ns_], ps2, x_res[:, fc, ns_],
                                scale=gbs_fc2[:, fc:fc + 1],
                                bias=gb_fc2[:, fc:fc + 1])
                            if nt == 1:
                                nc.sync.dma_start(out=out[fc * P:(fc + 1) * P, :],
                                                  in_=x_res[:, fc, :])
    nc.compile()
    return nc


_CACHE = {}


def _runner(nc, n_cores=8):
    import jax
    import numpy as _np
    from jax.sharding import Mesh, PartitionSpec, NamedSharding
    from jax.experimental.shard_map import shard_map
    from concourse.bass2jax import _bass_exec_p, install_neuronx_cc_hook, partition_id_tensor

    install_neuronx_cc_hook()
    in_names, out_names, out_avals = [], [], []
    partition_name = nc.partition_id_tensor.name if nc.partition_id_tensor else None
    for alloc in nc.m.functions[0].allocations:
        if not isinstance(alloc, mybir.MemoryLocationSet):
            continue
        nm = alloc.memorylocations[0].name
        if alloc.kind == "ExternalInput":
            if nm != partition_name:
                in_names.append(nm)
        elif alloc.kind == "ExternalOutput":
            out_names.append(nm)
            out_avals.append(jax.core.ShapedArray(tuple(alloc.tensor_shape),
                                                  mybir.dt.np(alloc.dtype)))

    def _body(*args):
        operands = list(args)
        if partition_name is not None:
            operands.append(partition_id_tensor())
        outs = _bass_exec_p.bind(
            *operands,
            out_avals=tuple(out_avals),
            in_names=tuple(in_names + [partition_name] if partition_name else in_names),
            out_names=tuple(out_names),
            lowering_input_output_aliases=(),
            sim_require_finite=False,
            sim_require_nnan=False,
            nc=nc,
        )
        return tuple(outs)

    devices = jax.devices()[:n_cores]
    mesh = Mesh(_np.asarray(devices), ("core",))
    fn = jax.jit(shard_map(_body, mesh=mesh,
                           in_specs=(PartitionSpec("core"),) * len(in_names),
                           out_specs=(PartitionSpec("core"),) * len(out_names),
                           check_rep=False))

    def run(in_maps):
        concat = [_np.concatenate([_np.asarray(m[n]) for m in in_maps], axis=0)
                  for n in in_names]
        args = [jax.device_put(c, NamedSharding(mesh, PartitionSpec("core")))
                for c in concat]
        outs = fn(*args)
        jax.block_until_ready(outs)
        res = []
        for c in range(n_cores):
            d = {}
            for i, nm in enumerate(out_names):
                full = _np.asarray(outs[i])
                d[nm] = full.reshape(n_cores, *out_avals[i].shape)[c]
            res.append(d)
        return res

    return run


def kernel(**inputs):
    """Full (unsharded) inputs -> full (B, T, D) float32 output."""
    if "nc" not in _CACHE:
        _CACHE["nc"] = build_dit(n_cores=8)
        _CACHE["run"] = _runner(_CACHE["nc"], 8)
    in_maps = host_prep(**inputs)
    results = _CACHE["run"](in_maps)
    return host_post(results)


# revision 38
# speedup vs baseline: 1.0089x; 1.0089x over previous
"""nn_DiTBlock on 8 TRN2 NeuronCores: data-parallel over batch (B=8), one
batch element per core. Self-contained: builds the Bass/Tile kernel, shards
inputs on the host (transpose/pack/cast only), runs SPMD via bass2jax/PJRT,
gathers and un-transposes the output.

v2 design: fp8e4(e4m3)+DoubleRow matmuls for qkv/v/attn@v/proj/fc1/fc2
(weights host-prescaled x128, descale folded into evictions), bf16 for adaLN
and attention scores, f32 residual + PSUM. exp scaled by 1/32 (cancels in
softmax); k-rmsnorm folded into the exp's per-partition scale; odd heads'
attn@v written directly to PSUM partitions 64:128 (no partition-move DMAs);
single-pass MLP; multi-chunk batched weight DMAs."""

import numpy as np
from contextlib import ExitStack

import concourse.bass as bass
import concourse.mybir as mybir
import concourse.tile as tile
from concourse import bacc


F32 = mybir.dt.float32
F32R = mybir.dt.float32r
BF16 = mybir.dt.bfloat16
FP8 = mybir.dt.float8e4
AF = mybir.ActivationFunctionType
OP = mybir.AluOpType
DR = mybir.MatmulPerfMode.DoubleRow

B, T, D, H = 8, 1024, 1024, 16
HD = D // H          # 64
DM = 4 * D           # 4096
NCH = D // 128       # 8
MCH = DM // 128      # 32
P = 128
WS = 128.0           # fp8 weight pre-scale (host)
ISV = 1.0 / WS
ELN32 = -3.4657359027997265  # -ln(32): exp pre-scale so fp8 es stays < 240


def host_prep(x, c, g1, g2, gq, gk, Wqkv, bqkv, Wproj, bproj,
              Wfc1, bfc1, Wfc2, bfc2, Wada, bada):
    import ml_dtypes
    E4 = mybir.dt.np(FP8)

    def packT(W, npdt, scale=1.0):  # (F, K) -> (K//128, 128, F) contiguous
        Wt = np.ascontiguousarray(np.asarray(W, np.float32).T * scale).astype(npdt)
        K, F = Wt.shape
        return np.ascontiguousarray(Wt.reshape(K // 128, 128, F))

    f32 = np.float32
    com = {
        "wqkv": packT(Wqkv, E4, WS), "wproj": packT(Wproj, E4, WS),
        "wfc1": packT(Wfc1, E4, WS), "wfc2": packT(Wfc2, E4, WS),
        "wada": packT(Wada, ml_dtypes.bfloat16),
        "bqkv": np.asarray(bqkv, f32), "bproj": np.asarray(bproj, f32),
        "bfc1": np.asarray(bfc1, f32), "bfc2": np.asarray(bfc2, f32),
        "bada": np.asarray(bada, f32),
        "g": np.stack([np.asarray(g1)[0], np.asarray(g2)[0],
                       np.asarray(gq)[0], np.asarray(gk)[0]]).astype(f32),
    }
    in_maps = []
    for b in range(B):
        m = dict(com)
        m["xt"] = np.ascontiguousarray(np.asarray(x[b], f32).T)
        m["cvec"] = np.asarray(c[b], f32)
        in_maps.append(m)
    return in_maps


def host_post(results):
    return np.ascontiguousarray(
        np.stack([r["out"].T for r in results]).astype(np.float32))


def col_ap(handle, nch):
    """DRAM (nch*128,) viewed as [128, nch]: tile[p, ch] = v[ch*128+p]."""
    return bass.AP(tensor=handle, offset=0, ap=[[1, P], [P, nch]])


def bc_ap(handle, n, offset=0):
    """DRAM (n,) broadcast-read to [128, n] (partition stride 0)."""
    return bass.AP(tensor=handle, offset=offset, ap=[[0, P], [1, n]])


def wload_ap(handle, kch, cols, col0):
    """DRAM weight pack [KCH,128,F] -> [128, kch, cols] AP at col offset."""
    F = handle.shape[2]
    return bass.AP(tensor=handle, offset=col0,
                   ap=[[F, P], [P * F, kch], [1, cols]])


def build_dit(n_cores=8):
    nc = bacc.Bacc("TRN2", target_bir_lowering=False, debug=False,
                   num_devices=n_cores)

    xt = nc.dram_tensor("xt", [D, T], F32, kind="ExternalInput")
    cin = nc.dram_tensor("cvec", [D], F32, kind="ExternalInput")
    g = nc.dram_tensor("g", [4], F32, kind="ExternalInput")
    wqkv = nc.dram_tensor("wqkv", [NCH, P, 3 * D], FP8, kind="ExternalInput")
    wproj = nc.dram_tensor("wproj", [NCH, P, D], FP8, kind="ExternalInput")
    wfc1 = nc.dram_tensor("wfc1", [NCH, P, DM], FP8, kind="ExternalInput")
    wfc2 = nc.dram_tensor("wfc2", [MCH, P, D], FP8, kind="ExternalInput")
    wada = nc.dram_tensor("wada", [NCH, P, 6 * D], BF16, kind="ExternalInput")
    bqkv = nc.dram_tensor("bqkv", [3 * D], F32, kind="ExternalInput")
    bproj = nc.dram_tensor("bproj", [D], F32, kind="ExternalInput")
    bfc1 = nc.dram_tensor("bfc1", [DM], F32, kind="ExternalInput")
    bfc2 = nc.dram_tensor("bfc2", [D], F32, kind="ExternalInput")
    bada = nc.dram_tensor("bada", [6 * D], F32, kind="ExternalInput")
    out = nc.dram_tensor("out", [D, T], F32, kind="ExternalOutput")

    with tile.TileContext(nc, pool_alloc_mode="queue") as tc:
        with ExitStack() as X:
            const = X.enter_context(tc.tile_pool(name="const", bufs=1))
            resid = X.enter_context(tc.tile_pool(name="resid", bufs=1))
            dram = X.enter_context(tc.tile_pool(name="dram", bufs=1, space="DRAM"))

            # ---------------- constants ----------------
            g_bc = const.tile([P, 4], F32)
            nc.sync.dma_start(out=g_bc, in_=bc_ap(g, 4))
            gsq = const.tile([P, 4], F32)
            nc.vector.tensor_tensor(gsq, g_bc, g_bc, OP.mult)
            ginv2 = const.tile([P, 4], F32)
            nc.vector.reciprocal(ginv2, gsq)
            # Rsqrt scales: rinv = rsqrt(ps * scl)
            scl_n1 = const.tile([P, 1], F32)
            nc.vector.tensor_scalar_mul(scl_n1, ginv2[:, 0:1], 1.0 / D)
            scl_n2 = const.tile([P, 1], F32)
            nc.vector.tensor_scalar_mul(scl_n2, ginv2[:, 1:2], 1.0 / D)
            scl_q = const.tile([P, 1], F32)
            nc.vector.tensor_copy(scl_q, ginv2[:, 2:3])
            scl_k = const.tile([P, 1], F32)
            nc.vector.tensor_scalar_mul(scl_k, ginv2[:, 3:4], 1.0 / HD)

            ones1_f = const.tile([P, 1], F32)
            nc.gpsimd.memset(ones1_f, 1.0)
            ones1 = const.tile([P, 1], BF16)
            nc.vector.tensor_copy(ones1, ones1_f)
            onesh_f = const.tile([P, 2], F32)
            nc.gpsimd.memset(onesh_f, 0.0)
            nc.gpsimd.memset(onesh_f[0:64, 0:1], 1.0)
            nc.gpsimd.memset(onesh_f[64:128, 1:2], 1.0)
            onesh = const.tile([P, 2], BF16)
            nc.vector.tensor_copy(onesh, onesh_f)

            bqkv_c = const.tile([P, 3 * D // P], F32)
            nc.sync.dma_start(out=bqkv_c, in_=col_ap(bqkv, 3 * D // P))
            bproj_c = const.tile([P, NCH], F32)
            nc.sync.dma_start(out=bproj_c, in_=col_ap(bproj, NCH))
            bfc1_c = const.tile([P, MCH], F32)
            nc.sync.dma_start(out=bfc1_c, in_=col_ap(bfc1, MCH))
            bfc2_c = const.tile([P, NCH], F32)
            nc.sync.dma_start(out=bfc2_c, in_=col_ap(bfc2, NCH))
            vbias_bc = const.tile([P, D], F32)
            nc.sync.dma_start(out=vbias_bc, in_=bc_ap(bqkv, D, offset=2 * D))
            eln32_c = const.tile([P, 1], F32)
            nc.gpsimd.memset(eln32_c, ELN32)

            x_res = resid.tile([P, NCH, T], F32)
            for j in range(4):
                nc.sync.dma_start(
                    out=x_res[:, 2 * j:2 * j + 2, :],
                    in_=bass.AP(tensor=xt, offset=2 * j * P * T,
                                ap=[[T, P], [P * T, 2], [1, T]]))

            c_pm = const.tile([P, NCH], F32)
            nc.sync.dma_start(out=c_pm, in_=col_ap(cin, NCH))
            cs_pm = const.tile([P, NCH], BF16)
            nc.scalar.activation(cs_pm, c_pm, AF.Silu)

            # ---------------- adaLN (bf16) ----------------
            ada_scr = dram.tile([1, 6 * D], F32)
            ada_sb = const.tile([1, 6 * D], F32)
            with tc.tile_pool(name="wadap", bufs=3) as wp, \
                 tc.tile_pool(name="psA", bufs=2, space="PSUM") as psA:
                for nb in range(12):
                    wt = wp.tile([P, NCH, 512], BF16, name="wt")
                    nc.sync.dma_start(out=wt, in_=wload_ap(wada, NCH, 512, nb * 512))
                    pa = psA.tile([1, 512], F32, name="pa")
                    for d in range(NCH):
                        nc.tensor.matmul(pa, cs_pm[:, d:d + 1], wt[:, d, :],
                                         start=(d == 0), stop=(d == NCH - 1))
                    nc.vector.tensor_copy(ada_sb[:, nb * 512:(nb + 1) * 512], pa)
            nc.sync.dma_start(out=ada_scr, in_=ada_sb)
            adaT = const.tile([P, 48], F32)
            nc.sync.dma_start(out=adaT, in_=bass.AP(tensor=ada_scr.tensor, offset=0,
                                                    ap=[[1, P], [P, 48]]))
            badaT = const.tile([P, 48], F32)
            nc.sync.dma_start(out=badaT, in_=col_ap(bada, 48))
            nc.vector.tensor_tensor(adaT, adaT, badaT, OP.add)
            # cols: shift_msa 0:8 | scale_msa 8:16 | gate_msa 16:24
            #       shift_mlp 24:32 | scale_mlp 32:40 | gate_mlp 40:48
            nc.vector.tensor_scalar_add(adaT[:, 8:16], adaT[:, 8:16], 1.0)
            nc.vector.tensor_scalar_add(adaT[:, 32:40], adaT[:, 32:40], 1.0)
            gb_proj = const.tile([P, NCH], F32)
            nc.vector.tensor_tensor(gb_proj, adaT[:, 16:24], bproj_c, OP.mult)
            gbs_proj = const.tile([P, NCH], F32)
            nc.vector.tensor_scalar_mul(gbs_proj, adaT[:, 16:24], ISV)
            gb_fc2 = const.tile([P, NCH], F32)
            nc.vector.tensor_tensor(gb_fc2, adaT[:, 40:48], bfc2_c, OP.mult)
            gbs_fc2 = const.tile([P, NCH], F32)
            nc.vector.tensor_scalar_mul(gbs_fc2, adaT[:, 40:48], ISV)

            def norm_modulate(scl, sh_col, sc_col, h_out):
                """x_res (f32) -> h_out (fp8): rmsnorm + adaLN modulate.
                Token-halved so the consumer can start on half 0 while the
                producer of x_res is still finishing half 1."""
                with tc.tile_pool(name="sqp", bufs=3) as sqp, \
                     tc.tile_pool(name="psN", bufs=1, space="PSUM") as psN, \
                     tc.tile_pool(name="nrm", bufs=2) as nrm, \
                     tc.tile_pool(name="xnp", bufs=3) as xnp:
                    pss = psN.tile([1, T], F32, name="pss")
                    for t2 in range(2):
                        ts_ = slice(t2 * 512, (t2 + 1) * 512)
                        for j in range(NCH):
                            xsq = sqp.tile([P, 512], BF16, name="xsq")
                            nc.scalar.activation(xsq, x_res[:, j, ts_], AF.Square)
                            nc.tensor.matmul(pss[:, ts_], ones1, xsq,
                                             start=(j == 0), stop=(j == NCH - 1))
                        rr = nrm.tile([1, 512], F32, name="rr")
                        nc.scalar.activation(rr, pss[:, ts_], AF.Sqrt,
                                             scale=scl[0:1, :])
                        rinv = nrm.tile([1, 512], F32, name="rinv")
                        nc.vector.reciprocal(rinv, rr)
                        rbc = nrm.tile([P, 512], F32, name="rbc")
                        nc.gpsimd.partition_broadcast(rbc, rinv)
                        for j in range(NCH):
                            xn = xnp.tile([P, 512], F32, name="xn")
                            nc.vector.tensor_tensor(xn, x_res[:, j, ts_], rbc,
                                                    OP.mult)
                            nc.gpsimd.tensor_scalar(h_out[:, j, ts_], xn,
                                                    adaT[:, sc_col + j:sc_col + j + 1],
                                                    adaT[:, sh_col + j:sh_col + j + 1],
                                                    OP.mult, OP.add)

            # fc1 weights tile created before the attention pools (so they
            # can close first); its load is emitted at proj time, landing
            # during attention when the wire is idle
            mlpw = X.enter_context(tc.tile_pool(name="mlpw", bufs=1))
            w1a = mlpw.tile([P, NCH, DM // 2], FP8)

            att = ExitStack()
            h1p = att.enter_context(tc.tile_pool(name="h1p", bufs=1, side="right"))
            h1 = h1p.tile([P, NCH, T], FP8)
            # ------------ norm1 + modulate ------------
            norm_modulate(scl_n1, 0, 8, h1)

            # ------------ q, k (feature-major bf16) + per-head rmsnorm ------------
            qp_ = att.enter_context(tc.tile_pool(name="qp_", bufs=1))
            kp_ = att.enter_context(tc.tile_pool(name="kp_", bufs=1))
            q_t = qp_.tile([P, NCH, T], BF16)
            k_t = kp_.tile([P, NCH, T], BF16)
            rkcp = att.enter_context(tc.tile_pool(name="rkcp", bufs=1))
            rkc = rkcp.tile([P, H, NCH], F32)  # 1/|k| per k-token, head-major

            # ------------ v (token-major fp8, ones-augmented) ------------
            # vx per-head 128-col slot: even h = [v(0:64) | ones@64 | 0],
            # odd h = [0 | ones@63 | v(64:128)]; attn@v DR outputs are then
            # always full [128, N] (walrus requires that) and odd heads land
            # on PSUM partitions 64:128 directly.
            vxp = att.enter_context(tc.tile_pool(name="vxp", bufs=1))
            vx = vxp.tile([P, NCH, H, P], FP8)   # [ktok][ktc][head][col]
            nc.gpsimd.memset(vx, 0.0)
            for h in range(H):
                oc = HD if h % 2 == 0 else 0
                nc.gpsimd.memset(vx[:, :, h, oc:oc + 1], 1.0)
            with tc.tile_pool(name="wvp", bufs=2) as wvp, \
                 tc.tile_pool(name="psV", bufs=3, space="PSUM") as psV:
                for nq in range(2):
                    wv = wvp.tile([P, NCH, 512], FP8, name="wv")
                    nc.sync.dma_start(out=wv,
                                      in_=wload_ap(wqkv, NCH, 512, 2 * D + nq * 512))
                    for t8 in range(NCH):
                        pv = psV.tile([P, 512], F32, name="pv")
                        for dp in range(4):
                            nc.tensor.matmul(
                                pv, h1[:, 2 * dp:2 * dp + 2, t8 * P:(t8 + 1) * P],
                                wv[:, 2 * dp:2 * dp + 2, :],
                                start=(dp == 0), stop=(dp == 3), perf_mode=DR)
                        # heads alternate col-base 0 (even) / 64 (odd) in vx
                        vblk = vx[:, t8, :, :].rearrange(
                            "p h c -> p (h c)").rearrange(
                            "p (i r) -> p i r", r=256)  # [P, 8, 256]
                        for par in range(2):
                            nc.vector.scalar_tensor_tensor(
                                vblk[:, 4 * nq:4 * nq + 4,
                                     192 * par:192 * par + HD],
                                pv.rearrange("p (i r) -> p i r", r=128)[
                                    :, :, par * HD:(par + 1) * HD], ISV,
                                vbias_bc[:, nq * 512:(nq + 1) * 512].rearrange(
                                    "p (i r) -> p i r", r=128)[
                                    :, :, par * HD:(par + 1) * HD],
                                OP.mult, OP.add)

            with tc.tile_pool(name="wqp", bufs=2) as wqp, \
                 tc.tile_pool(name="sqq", bufs=2) as sqq, \
                 tc.tile_pool(name="psD", bufs=2, space="PSUM") as psD, \
                 tc.tile_pool(name="psR", bufs=1, space="PSUM") as psR, \
                 tc.tile_pool(name="nrq", bufs=1) as nrq:
                def finish_q(ch, tiles):
                    for hfq in range(2):
                        prh = tiles[hfq]
                        rr2 = nrq.tile([1, T], BF16, name=f"rr2{hfq}")
                        nc.scalar.activation(rr2, prh, AF.Sqrt,
                                             scale=scl_q[0:1, :])
                        rinv_sb = nrq.tile([1, T], BF16, name=f"ri{hfq}")
                        with nc.allow_low_precision(reason="1/|q| bf16"):
                            nc.vector.reciprocal(rinv_sb, rr2)
                        rbcq = nrq.tile([P, T], BF16, name=f"rbcq{hfq}")
                        nc.gpsimd.partition_broadcast(rbcq, rinv_sb)
                        hs = slice(64 * hfq, 64 * (hfq + 1))
                        nc.vector.tensor_tensor(q_t[hs, ch, :], q_t[hs, ch, :],
                                                rbcq[hs, :], OP.mult)

                def finish_k(fc, tiles):
                    for j in range(2):
                        hidx = 2 * (fc - 8) + j
                        rrk = nrq.tile([P, NCH], F32, name="rrk")
                        nc.scalar.activation(rrk, tiles[j], AF.Sqrt, scale=scl_k)
                        nc.vector.reciprocal(rkc[:, hidx, :], rrk)

                pending = None
                for fc in range(16):  # q: 0..7, k: 8..15
                    if fc % 4 == 0:
                        wt = wqp.tile([P, NCH, 512], FP8, name="wt")
                        nc.sync.dma_start(out=wt, in_=wload_ap(wqkv, NCH, 512, fc * P))
                    tgt = q_t if fc < 8 else k_t
                    ch = fc % 8
                    ps = [psD.tile([P, 512], F32, name="ps") for _ in range(2)]
                    for nt in range(2):
                        for dp in range(4):
                            nc.tensor.matmul(
                                ps[nt],
                                wt[:, 2 * dp:2 * dp + 2, (fc % 4) * P:(fc % 4 + 1) * P],
                                h1[:, 2 * dp:2 * dp + 2, nt * 512:(nt + 1) * 512],
                                start=(dp == 0), stop=(dp == 3), perf_mode=DR)
                        # evict: (psum/128 + bias) -> bf16 on Act (idle here)
                        nc.scalar.activation(tgt[:, ch, nt * 512:(nt + 1) * 512],
                                             ps[nt], AF.Identity, scale=ISV,
                                             bias=bqkv_c[:, fc:fc + 1])
                    # run the previous chunk's norm-finisher here so its Act
                    # Sqrt never heads the queue before its deps are ready
                    if pending is not None:
                        pending()
                    # sum of squares per head
                    sq = sqq.tile([P, T], BF16, name="sq")
                    nc.vector.tensor_tensor(sq, tgt[:, ch, :], tgt[:, ch, :], OP.mult)
                    if fc < 8:
                        tiles = []
                        for hfq in range(2):
                            prh = psR.tile([1, T], F32, name=f"prh{hfq}")
                            for nt in range(2):
                                nc.tensor.matmul(
                                    prh[:, nt * 512:(nt + 1) * 512],
                                    onesh[:, hfq:hfq + 1],
                                    sq[:, nt * 512:(nt + 1) * 512],
                                    start=True, stop=True)
                            tiles.append(prh)
                        pending = (lambda c=ch, t=tiles: finish_q(c, t))
                    else:
                        tiles = []
                        for j in range(2):
                            pkn = psR.tile([P, NCH], F32, name=f"pkn{j}")
                            for kt in range(NCH):
                                nc.tensor.matmul(
                                    pkn[:, kt:kt + 1],
                                    sq[64 * j:64 * (j + 1), kt * P:(kt + 1) * P],
                                    ones1[64 * j:64 * (j + 1), :],
                                    start=True, stop=True)
                            tiles.append(pkn)
                        pending = (lambda f=fc, t=tiles: finish_k(f, t))
                pending()

            # ------------ attention ------------
            oTp = att.enter_context(tc.tile_pool(name="oTp", bufs=1, side="right"))
            oT = oTp.tile([P, NCH, T], FP8)
            with tc.tile_pool(name="esp", bufs=2) as esp, \
                 tc.tile_pool(name="psS", bufs=2, space="PSUM") as psS, \
                 tc.tile_pool(name="psO", bufs=3, space="PSUM") as psO, \
                 tc.tile_pool(name="onp", bufs=4) as onp:
                for h in range(H):
                    hc, hf = h // 2, h % 2
                    rq = slice(64 * hf, 64 * (hf + 1))
                    es_h = esp.tile([P, NCH, T], FP8, name="es")
                    for ktc in range(NCH):
                        psc = psS.tile([P, T], F32, name="psc")
                        for qt in range(2):
                            nc.tensor.matmul(psc[:, qt * 512:(qt + 1) * 512],
                                             k_t[rq, hc, ktc * P:(ktc + 1) * P],
                                             q_t[rq, hc, qt * 512:(qt + 1) * 512],
                                             start=True, stop=True)
                        nc.scalar.activation(es_h[:, ktc, :], psc, AF.Exp,
                                             bias=eln32_c, scale=rkc[:, h, ktc:ktc + 1])
                    for qt in range(2):
                        qs = slice(qt * 512, (qt + 1) * 512)
                        po = psO.tile([P, 512], F32, name="po")
                        rs = onp.tile([P, 512], F32, name="rs")
                        rsb = onp.tile([P, 512], F32, name="rsb")
                        for kp in range(4):
                            nc.tensor.matmul(
                                po, vx[:, 2 * kp:2 * kp + 2, h, :],
                                es_h[:, 2 * kp:2 * kp + 2, qs],
                                start=(kp == 0), stop=(kp == 3), perf_mode=DR)
                        if hf == 0:
                            # denom at row 64: recip there, DMA row to
                            # partition 0, broadcast full, use rows 0:64
                            nc.vector.reciprocal(rs[64:65, :], po[64:65, :])
                            rse = onp.tile([1, 512], F32, name="rse")
                            nc.sync.dma_start(out=rse, in_=rs[64:65, :])
                            nc.gpsimd.partition_broadcast(rsb, rse)
                            nc.vector.tensor_tensor(oT[0:64, hc, qs], po[0:64, :],
                                                    rsb[0:64, :], OP.mult)
                        else:
                            # denom at row 0: broadcast full, use rows 64:128
                            nc.vector.reciprocal(rs[0:1, :], po[0:1, :])
                            nc.gpsimd.partition_broadcast(rsb, rs[0:1, :])
                            nc.vector.tensor_tensor(oT[64:128, hc, qs], po[64:128, :],
                                                    rsb[64:128, :], OP.mult)

            # ------------ proj + residual ------------
            with tc.tile_pool(name="wpp", bufs=1) as wpp, \
                 tc.tile_pool(name="psP", bufs=3, space="PSUM") as psP:
                wpj = wpp.tile([P, NCH, D], FP8, name="wpj")
                nc.sync.dma_start(out=wpj, in_=wload_ap(wproj, NCH, D, 0))
                nc.sync.dma_start(out=w1a, in_=wload_ap(wfc1, NCH, DM // 2, 0))
                for nt in range(2):
                    for fc in range(8):
                        pp = psP.tile([P, 512], F32, name="pp")
                        for dp in range(4):
                            nc.tensor.matmul(
                                pp, wpj[:, 2 * dp:2 * dp + 2, fc * P:(fc + 1) * P],
                                oT[:, 2 * dp:2 * dp + 2, nt * 512:(nt + 1) * 512],
                                start=(dp == 0), stop=(dp == 3), perf_mode=DR)
                        nc.vector.affine_then_add(
                            x_res[:, fc, nt * 512:(nt + 1) * 512], pp,
                            x_res[:, fc, nt * 512:(nt + 1) * 512],
                            scale=gbs_proj[:, fc:fc + 1],
                            bias=gb_proj[:, fc:fc + 1])

            att.close()  # free h1, q/k, vx, oT, rkc

            # ------------ norm2 + modulate + MLP (single pass, fp8) ------------
            with tc.tile_pool(name="h2p", bufs=1) as h2p, \
                 tc.tile_pool(name="gactp", bufs=1, side="right") as gactp:
                h2 = h2p.tile([P, NCH, T], FP8)
                norm_modulate(scl_n2, 24, 32, h2)
                gact = gactp.tile([P, MCH, T], FP8)
                w1b = gactp.tile([P, NCH, DM // 2], FP8, name="w1b")
                nc.sync.dma_start(out=w1b, in_=wload_ap(wfc1, NCH, DM // 2, DM // 2))
                w2 = gactp.tile([P, MCH, D], FP8, name="w2full")
                nc.sync.dma_start(out=w2, in_=wload_ap(wfc2, MCH, D, 0))
                # nt-outer: fc2 on token-half 0 overlaps fc1/gelu on half 1
                with tc.tile_pool(name="psM", bufs=3, space="PSUM") as psM, \
                     tc.tile_pool(name="psM2", bufs=3, space="PSUM") as psM2:
                    for nt in range(2):
                        ns_ = slice(nt * 512, (nt + 1) * 512)
                        for m in range(MCH):
                            psm = psM.tile([P, 512], F32, name="psm")
                            w1h = w1a if m < 16 else w1b
                            mo = m if m < 16 else m - 16
                            for dp in range(4):
                                nc.tensor.matmul(
                                    psm,
                                    w1h[:, 2 * dp:2 * dp + 2, mo * P:(mo + 1) * P],
                                    h2[:, 2 * dp:2 * dp + 2, ns_],
                                    start=(dp == 0), stop=(dp == 3), perf_mode=DR)
                            nc.scalar.activation(gact[:, m, ns_], psm,
                                                 AF.Gelu_apprx_tanh, scale=ISV,
                                                 bias=bfc1_c[:, m:m + 1])
                        for fc in range(8):
                            ps2 = psM2.tile([P, 512], F32, name="ps2")
                            for dp in range(16):
                                nc.tensor.matmul(
                                    ps2,
                                    w2[:, 2 * dp:2 * dp + 2, fc * P:(fc + 1) * P],
                                    gact[:, 2 * dp:2 * dp + 2, ns_],
                                    start=(dp == 0), stop=(dp == 15), perf_mode=DR)
                            nc.vector.affine_then_add(
                                x_res[:, fc, ns_], ps2, x_res[:, fc, ns_],
                                scale=gbs_fc2[:, fc:fc + 1],
                                bias=gb_fc2[:, fc:fc + 1])
                            if nt == 1:
                                nc.sync.dma_start(out=out[fc * P:(fc + 1) * P, :],
                                                  in_=x_res[:, fc, :])
    nc.compile()
    return nc


_CACHE = {}


def _runner(nc, n_cores=8):
    import jax
    import numpy as _np
    from jax.sharding import Mesh, PartitionSpec, NamedSharding
    from jax.experimental.shard_map import shard_map
    from concourse.bass2jax import _bass_exec_p, install_neuronx_cc_hook, partition_id_tensor

    install_neuronx_cc_hook()
    in_names, out_names, out_avals = [], [], []
    partition_name = nc.partition_id_tensor.name if nc.partition_id_tensor else None
    for alloc in nc.m.functions[0].allocations:
        if not isinstance(alloc, mybir.MemoryLocationSet):
            continue
        nm = alloc.memorylocations[0].name
        if alloc.kind == "ExternalInput":
            if nm != partition_name:
                in_names.append(nm)
        elif alloc.kind == "ExternalOutput":
            out_names.append(nm)
            out_avals.append(jax.core.ShapedArray(tuple(alloc.tensor_shape),
                                                  mybir.dt.np(alloc.dtype)))

    def _body(*args):
        operands = list(args)
        if partition_name is not None:
            operands.append(partition_id_tensor())
        outs = _bass_exec_p.bind(
            *operands,
            out_avals=tuple(out_avals),
            in_names=tuple(in_names + [partition_name] if partition_name else in_names),
            out_names=tuple(out_names),
            lowering_input_output_aliases=(),
            sim_require_finite=False,
            sim_require_nnan=False,
            nc=nc,
        )
        return tuple(outs)

    devices = jax.devices()[:n_cores]
    mesh = Mesh(_np.asarray(devices), ("core",))
    fn = jax.jit(shard_map(_body, mesh=mesh,
                           in_specs=(PartitionSpec("core"),) * len(in_names),
                           out_specs=(PartitionSpec("core"),) * len(out_names),
                           check_rep=False))

    def run(in_maps):
        concat = [_np.concatenate([_np.asarray(m[n]) for m in in_maps], axis=0)
                  for n in in_names]
        args = [jax.device_put(c, NamedSharding(mesh, PartitionSpec("core")))
                for c in concat]
        outs = fn(*args)
        jax.block_until_ready(outs)
        res = []
        for c in range(n_cores):
            d = {}
            for i, nm in enumerate(out_names):
                full = _np.asarray(outs[i])
                d[nm] = full.reshape(n_cores, *out_avals[i].shape)[c]
            res.append(d)
        return res

    return run


def kernel(**inputs):
    """Full (unsharded) inputs -> full (B, T, D) float32 output."""
    if "nc" not in _CACHE:
        _CACHE["nc"] = build_dit(n_cores=8)
        _CACHE["run"] = _runner(_CACHE["nc"], 8)
    in_maps = host_prep(**inputs)
    results = _CACHE["run"](in_maps)
    return host_post(results)


# revision 44
# speedup vs baseline: 1.0250x; 1.0159x over previous
"""nn_DiTBlock on 8 TRN2 NeuronCores: data-parallel over batch (B=8), one
batch element per core. Self-contained: builds the Bass/Tile kernel, shards
inputs on the host (transpose/pack/cast only), runs SPMD via bass2jax/PJRT,
gathers and un-transposes the output.

v2 design: fp8e4(e4m3)+DoubleRow matmuls for qkv/v/attn@v/proj/fc1/fc2
(weights host-prescaled x128, descale folded into evictions), bf16 for adaLN
and attention scores, f32 residual + PSUM. exp scaled by 1/32 (cancels in
softmax); k-rmsnorm folded into the exp's per-partition scale; odd heads'
attn@v written directly to PSUM partitions 64:128 (no partition-move DMAs);
single-pass MLP; multi-chunk batched weight DMAs."""

import numpy as np
from contextlib import ExitStack

import concourse.bass as bass
import concourse.mybir as mybir
import concourse.tile as tile
from concourse import bacc


F32 = mybir.dt.float32
F32R = mybir.dt.float32r
BF16 = mybir.dt.bfloat16
FP8 = mybir.dt.float8e4
AF = mybir.ActivationFunctionType
OP = mybir.AluOpType
DR = mybir.MatmulPerfMode.DoubleRow

B, T, D, H = 8, 1024, 1024, 16
HD = D // H          # 64
DM = 4 * D           # 4096
NCH = D // 128       # 8
MCH = DM // 128      # 32
P = 128
WS = 128.0           # fp8 weight pre-scale (host)
ISV = 1.0 / WS
ELN32 = -3.4657359027997265  # -ln(32): exp pre-scale so fp8 es stays < 240


def host_prep(x, c, g1, g2, gq, gk, Wqkv, bqkv, Wproj, bproj,
              Wfc1, bfc1, Wfc2, bfc2, Wada, bada):
    import ml_dtypes
    E4 = mybir.dt.np(FP8)

    def packT(W, npdt, scale=1.0):  # (F, K) -> (K//128, 128, F) contiguous
        Wt = np.ascontiguousarray(np.asarray(W, np.float32).T * scale).astype(npdt)
        K, F = Wt.shape
        return np.ascontiguousarray(Wt.reshape(K // 128, 128, F))

    f32 = np.float32
    com = {
        "wqkv": packT(Wqkv, E4, WS), "wproj": packT(Wproj, E4, WS),
        "wfc1": packT(Wfc1, E4, WS), "wfc2": packT(Wfc2, E4, WS),
        "wada8": packT(np.asarray(Wada, np.float32)[
            np.r_[0:2048, 3072:5120], :], E4, WS),
        "wadab": packT(np.asarray(Wada, np.float32)[
            np.r_[2048:3072, 5120:6144], :], ml_dtypes.bfloat16),
        "bqkv": np.asarray(bqkv, f32), "bproj": np.asarray(bproj, f32),
        "bfc1": np.asarray(bfc1, f32), "bfc2": np.asarray(bfc2, f32),
        "bada": np.asarray(bada, f32),
        "g": np.stack([np.asarray(g1)[0], np.asarray(g2)[0],
                       np.asarray(gq)[0], np.asarray(gk)[0]]).astype(f32),
    }
    in_maps = []
    for b in range(B):
        m = dict(com)
        m["xt"] = np.ascontiguousarray(np.asarray(x[b], f32).T)
        m["cvec"] = np.asarray(c[b], f32)
        in_maps.append(m)
    return in_maps


def host_post(results):
    return np.ascontiguousarray(
        np.stack([r["out"].T for r in results]).astype(np.float32))


def col_ap(handle, nch):
    """DRAM (nch*128,) viewed as [128, nch]: tile[p, ch] = v[ch*128+p]."""
    return bass.AP(tensor=handle, offset=0, ap=[[1, P], [P, nch]])


def bc_ap(handle, n, offset=0):
    """DRAM (n,) broadcast-read to [128, n] (partition stride 0)."""
    return bass.AP(tensor=handle, offset=offset, ap=[[0, P], [1, n]])


def wload_ap(handle, kch, cols, col0):
    """DRAM weight pack [KCH,128,F] -> [128, kch, cols] AP at col offset."""
    F = handle.shape[2]
    return bass.AP(tensor=handle, offset=col0,
                   ap=[[F, P], [P * F, kch], [1, cols]])


def build_dit(n_cores=8):
    nc = bacc.Bacc("TRN2", target_bir_lowering=False, debug=False,
                   num_devices=n_cores)

    xt = nc.dram_tensor("xt", [D, T], F32, kind="ExternalInput")
    cin = nc.dram_tensor("cvec", [D], F32, kind="ExternalInput")
    g = nc.dram_tensor("g", [4], F32, kind="ExternalInput")
    wqkv = nc.dram_tensor("wqkv", [NCH, P, 3 * D], FP8, kind="ExternalInput")
    wproj = nc.dram_tensor("wproj", [NCH, P, D], FP8, kind="ExternalInput")
    wfc1 = nc.dram_tensor("wfc1", [NCH, P, DM], FP8, kind="ExternalInput")
    wfc2 = nc.dram_tensor("wfc2", [MCH, P, D], FP8, kind="ExternalInput")
    wada8 = nc.dram_tensor("wada8", [NCH, P, 4 * D], FP8, kind="ExternalInput")
    wadab = nc.dram_tensor("wadab", [NCH, P, 2 * D], BF16, kind="ExternalInput")
    bqkv = nc.dram_tensor("bqkv", [3 * D], F32, kind="ExternalInput")
    bproj = nc.dram_tensor("bproj", [D], F32, kind="ExternalInput")
    bfc1 = nc.dram_tensor("bfc1", [DM], F32, kind="ExternalInput")
    bfc2 = nc.dram_tensor("bfc2", [D], F32, kind="ExternalInput")
    bada = nc.dram_tensor("bada", [6 * D], F32, kind="ExternalInput")
    out = nc.dram_tensor("out", [D, T], F32, kind="ExternalOutput")

    with tile.TileContext(nc, pool_alloc_mode="queue") as tc:
        with ExitStack() as X:
            const = X.enter_context(tc.tile_pool(name="const", bufs=1))
            resid = X.enter_context(tc.tile_pool(name="resid", bufs=1))
            dram = X.enter_context(tc.tile_pool(name="dram", bufs=1, space="DRAM"))

            # ---------------- constants ----------------
            g_bc = const.tile([P, 4], F32)
            nc.sync.dma_start(out=g_bc, in_=bc_ap(g, 4))
            gsq = const.tile([P, 4], F32)
            nc.vector.tensor_tensor(gsq, g_bc, g_bc, OP.mult)
            ginv2 = const.tile([P, 4], F32)
            nc.vector.reciprocal(ginv2, gsq)
            # Rsqrt scales: rinv = rsqrt(ps * scl)
            scl_n1 = const.tile([P, 1], F32)
            nc.vector.tensor_scalar_mul(scl_n1, ginv2[:, 0:1], 1.0 / D)
            scl_n2 = const.tile([P, 1], F32)
            nc.vector.tensor_scalar_mul(scl_n2, ginv2[:, 1:2], 1.0 / D)
            scl_q = const.tile([P, 1], F32)
            nc.vector.tensor_copy(scl_q, ginv2[:, 2:3])
            scl_k = const.tile([P, 1], F32)
            nc.vector.tensor_scalar_mul(scl_k, ginv2[:, 3:4], 1.0 / HD)

            ones1_f = const.tile([P, 1], F32)
            nc.gpsimd.memset(ones1_f, 1.0)
            ones1 = const.tile([P, 1], BF16)
            nc.vector.tensor_copy(ones1, ones1_f)
            onesh_f = const.tile([P, 2], F32)
            nc.gpsimd.memset(onesh_f, 0.0)
            nc.gpsimd.memset(onesh_f[0:64, 0:1], 1.0)
            nc.gpsimd.memset(onesh_f[64:128, 1:2], 1.0)
            onesh = const.tile([P, 2], BF16)
            nc.vector.tensor_copy(onesh, onesh_f)

            bqkv_c = const.tile([P, 3 * D // P], F32)
            nc.sync.dma_start(out=bqkv_c, in_=col_ap(bqkv, 3 * D // P))
            bproj_c = const.tile([P, NCH], F32)
            nc.sync.dma_start(out=bproj_c, in_=col_ap(bproj, NCH))
            bfc1_c = const.tile([P, MCH], F32)
            nc.sync.dma_start(out=bfc1_c, in_=col_ap(bfc1, MCH))
            bfc2_c = const.tile([P, NCH], F32)
            nc.sync.dma_start(out=bfc2_c, in_=col_ap(bfc2, NCH))
            vbias_bc = const.tile([P, D], F32)
            nc.sync.dma_start(out=vbias_bc, in_=bc_ap(bqkv, D, offset=2 * D))
            eln32_c = const.tile([P, 1], F32)
            nc.gpsimd.memset(eln32_c, ELN32)

            x_res = resid.tile([P, NCH, T], F32)
            for j in range(4):
                nc.sync.dma_start(
                    out=x_res[:, 2 * j:2 * j + 2, :],
                    in_=bass.AP(tensor=xt, offset=2 * j * P * T,
                                ap=[[T, P], [P * T, 2], [1, T]]))

            c_pm = const.tile([P, NCH], F32)
            nc.sync.dma_start(out=c_pm, in_=col_ap(cin, NCH))
            cs_pm = const.tile([P, NCH], BF16)
            nc.scalar.activation(cs_pm, c_pm, AF.Silu)

            # ---------------- adaLN (bf16) ----------------
            ada_scr = dram.tile([1, 6 * D], F32)
            ada_sb = const.tile([1, 6 * D], F32)
            cs8 = const.tile([P, NCH], FP8)
            nc.vector.tensor_copy(cs8, cs_pm)
            with tc.tile_pool(name="wadap", bufs=3) as wp, \
                 tc.tile_pool(name="psA", bufs=2, space="PSUM") as psA:
                # shift/scale blocks in fp8 (x128), original col offsets:
                # wada8 = [sh_msa|sc_msa|sh_mlp|sc_mlp], wadab = [g_msa|g_mlp]
                for nb in range(8):
                    og = nb * 512 if nb < 4 else 3072 + (nb - 4) * 512
                    wt8 = wp.tile([P, NCH, 512], FP8, name="wt8")
                    nc.sync.dma_start(out=wt8, in_=wload_ap(wada8, NCH, 512, nb * 512))
                    pa = psA.tile([1, 512], F32, name="pa")
                    for d in range(NCH):
                        nc.tensor.matmul(pa, cs8[:, d:d + 1], wt8[:, d, :],
                                         start=(d == 0), stop=(d == NCH - 1))
                    nc.vector.tensor_scalar_mul(ada_sb[:, og:og + 512], pa, ISV)
                for nb in range(8):
                    og = (2048 + nb * 256) if nb < 4 else (5120 + (nb - 4) * 256)
                    wt = wp.tile([P, NCH, 256], BF16, name="wt")
                    nc.sync.dma_start(out=wt, in_=wload_ap(wadab, NCH, 256, nb * 256))
                    pa = psA.tile([1, 256], F32, name="pab")
                    for d in range(NCH):
                        nc.tensor.matmul(pa, cs_pm[:, d:d + 1], wt[:, d, :],
                                         start=(d == 0), stop=(d == NCH - 1))
                    nc.vector.tensor_copy(ada_sb[:, og:og + 256], pa)
            nc.sync.dma_start(out=ada_scr, in_=ada_sb)
            adaT = const.tile([P, 48], F32)
            nc.sync.dma_start(out=adaT, in_=bass.AP(tensor=ada_scr.tensor, offset=0,
                                                    ap=[[1, P], [P, 48]]))
            badaT = const.tile([P, 48], F32)
            nc.sync.dma_start(out=badaT, in_=col_ap(bada, 48))
            nc.vector.tensor_tensor(adaT, adaT, badaT, OP.add)
            # cols: shift_msa 0:8 | scale_msa 8:16 | gate_msa 16:24
            #       shift_mlp 24:32 | scale_mlp 32:40 | gate_mlp 40:48
            nc.vector.tensor_scalar_add(adaT[:, 8:16], adaT[:, 8:16], 1.0)
            nc.vector.tensor_scalar_add(adaT[:, 32:40], adaT[:, 32:40], 1.0)
            gb_proj = const.tile([P, NCH], F32)
            nc.vector.tensor_tensor(gb_proj, adaT[:, 16:24], bproj_c, OP.mult)
            gbs_proj = const.tile([P, NCH], F32)
            nc.vector.tensor_scalar_mul(gbs_proj, adaT[:, 16:24], ISV)
            gb_fc2 = const.tile([P, NCH], F32)
            nc.vector.tensor_tensor(gb_fc2, adaT[:, 40:48], bfc2_c, OP.mult)
            gbs_fc2 = const.tile([P, NCH], F32)
            nc.vector.tensor_scalar_mul(gbs_fc2, adaT[:, 40:48], ISV)

            def norm_modulate(scl, sh_col, sc_col, h_out):
                """x_res (f32) -> h_out (fp8): rmsnorm + adaLN modulate.
                Token-halved so the consumer can start on half 0 while the
                producer of x_res is still finishing half 1."""
                with tc.tile_pool(name="sqp", bufs=3) as sqp, \
                     tc.tile_pool(name="psN", bufs=1, space="PSUM") as psN, \
                     tc.tile_pool(name="nrm", bufs=2) as nrm, \
                     tc.tile_pool(name="xnp", bufs=3) as xnp:
                    pss = psN.tile([1, T], F32, name="pss")
                    for t2 in range(2):
                        ts_ = slice(t2 * 512, (t2 + 1) * 512)
                        for j in range(NCH):
                            xsq = sqp.tile([P, 512], BF16, name="xsq")
                            nc.scalar.activation(xsq, x_res[:, j, ts_], AF.Square)
                            nc.tensor.matmul(pss[:, ts_], ones1, xsq,
                                             start=(j == 0), stop=(j == NCH - 1))
                        rr = nrm.tile([1, 512], F32, name="rr")
                        nc.scalar.activation(rr, pss[:, ts_], AF.Sqrt,
                                             scale=scl[0:1, :])
                        rinv = nrm.tile([1, 512], F32, name="rinv")
                        nc.vector.reciprocal(rinv, rr)
                        rbc = nrm.tile([P, 512], F32, name="rbc")
                        nc.gpsimd.partition_broadcast(rbc, rinv)
                        for j in range(NCH):
                            xn = xnp.tile([P, 512], F32, name="xn")
                            nc.vector.tensor_tensor(xn, x_res[:, j, ts_], rbc,
                                                    OP.mult)
                            nc.gpsimd.tensor_scalar(h_out[:, j, ts_], xn,
                                                    adaT[:, sc_col + j:sc_col + j + 1],
                                                    adaT[:, sh_col + j:sh_col + j + 1],
                                                    OP.mult, OP.add)

            # fc1 weights tile created before the attention pools (so they
            # can close first); its load is emitted at proj time, landing
            # during attention when the wire is idle
            mlpw = X.enter_context(tc.tile_pool(name="mlpw", bufs=1))
            w1a = mlpw.tile([P, NCH, DM // 2], FP8)

            att = ExitStack()
            h1p = att.enter_context(tc.tile_pool(name="h1p", bufs=1, side="right"))
            h1 = h1p.tile([P, NCH, T], FP8)
            # ------------ norm1 + modulate ------------
            norm_modulate(scl_n1, 0, 8, h1)

            # ------------ q, k (feature-major bf16) + per-head rmsnorm ------------
            qp_ = att.enter_context(tc.tile_pool(name="qp_", bufs=1))
            kp_ = att.enter_context(tc.tile_pool(name="kp_", bufs=1))
            q_t = qp_.tile([P, NCH, T], BF16)
            k_t = kp_.tile([P, NCH, T], BF16)
            rkcp = att.enter_context(tc.tile_pool(name="rkcp", bufs=1))
            rkc = rkcp.tile([P, H, NCH], F32)  # 1/|k| per k-token, head-major

            # ------------ v (token-major fp8, ones-augmented) ------------
            # vx per-head 128-col slot: even h = [v(0:64) | ones@64 | 0],
            # odd h = [0 | ones@63 | v(64:128)]; attn@v DR outputs are then
            # always full [128, N] (walrus requires that) and odd heads land
            # on PSUM partitions 64:128 directly.
            vxp = att.enter_context(tc.tile_pool(name="vxp", bufs=1))
            vx = vxp.tile([P, NCH, H, P], FP8)   # [ktok][ktc][head][col]
            nc.gpsimd.memset(vx, 0.0)
            for h in range(H):
                oc = HD if h % 2 == 0 else 0
                nc.gpsimd.memset(vx[:, :, h, oc:oc + 1], 1.0)
            with tc.tile_pool(name="wvp", bufs=2) as wvp, \
                 tc.tile_pool(name="psV", bufs=3, space="PSUM") as psV:
                for nq in range(2):
                    wv = wvp.tile([P, NCH, 512], FP8, name="wv")
                    nc.sync.dma_start(out=wv,
                                      in_=wload_ap(wqkv, NCH, 512, 2 * D + nq * 512))
                    for t8 in range(NCH):
                        pv = psV.tile([P, 512], F32, name="pv")
                        for dp in range(4):
                            nc.tensor.matmul(
                                pv, h1[:, 2 * dp:2 * dp + 2, t8 * P:(t8 + 1) * P],
                                wv[:, 2 * dp:2 * dp + 2, :],
                                start=(dp == 0), stop=(dp == 3), perf_mode=DR)
                        # heads alternate col-base 0 (even) / 64 (odd) in vx
                        vblk = vx[:, t8, :, :].rearrange(
                            "p h c -> p (h c)").rearrange(
                            "p (i r) -> p i r", r=256)  # [P, 8, 256]
                        for par in range(2):
                            nc.vector.scalar_tensor_tensor(
                                vblk[:, 4 * nq:4 * nq + 4,
                                     192 * par:192 * par + HD],
                                pv.rearrange("p (i r) -> p i r", r=128)[
                                    :, :, par * HD:(par + 1) * HD], ISV,
                                vbias_bc[:, nq * 512:(nq + 1) * 512].rearrange(
                                    "p (i r) -> p i r", r=128)[
                                    :, :, par * HD:(par + 1) * HD],
                                OP.mult, OP.add)

            with tc.tile_pool(name="wqp", bufs=2) as wqp, \
                 tc.tile_pool(name="sqq", bufs=2) as sqq, \
                 tc.tile_pool(name="psD", bufs=2, space="PSUM") as psD, \
                 tc.tile_pool(name="psR", bufs=1, space="PSUM") as psR, \
                 tc.tile_pool(name="nrq", bufs=1) as nrq:
                def finish_q(ch, tiles):
                    for hfq in range(2):
                        prh = tiles[hfq]
                        rr2 = nrq.tile([1, T], BF16, name=f"rr2{hfq}")
                        nc.scalar.activation(rr2, prh, AF.Sqrt,
                                             scale=scl_q[0:1, :])
                        rinv_sb = nrq.tile([1, T], BF16, name=f"ri{hfq}")
                        with nc.allow_low_precision(reason="1/|q| bf16"):
                            nc.vector.reciprocal(rinv_sb, rr2)
                        rbcq = nrq.tile([P, T], BF16, name=f"rbcq{hfq}")
                        nc.gpsimd.partition_broadcast(rbcq, rinv_sb)
                        hs = slice(64 * hfq, 64 * (hfq + 1))
                        nc.vector.tensor_tensor(q_t[hs, ch, :], q_t[hs, ch, :],
                                                rbcq[hs, :], OP.mult)

                def finish_k(fc, tiles):
                    for j in range(2):
                        hidx = 2 * (fc - 8) + j
                        rrk = nrq.tile([P, NCH], F32, name="rrk")
                        nc.scalar.activation(rrk, tiles[j], AF.Sqrt, scale=scl_k)
                        nc.vector.reciprocal(rkc[:, hidx, :], rrk)

                pending = None
                for fc in range(16):  # q: 0..7, k: 8..15
                    if fc % 4 == 0:
                        wt = wqp.tile([P, NCH, 512], FP8, name="wt")
                        nc.sync.dma_start(out=wt, in_=wload_ap(wqkv, NCH, 512, fc * P))
                    tgt = q_t if fc < 8 else k_t
                    ch = fc % 8
                    ps = [psD.tile([P, 512], F32, name="ps") for _ in range(2)]
                    for nt in range(2):
                        for dp in range(4):
                            nc.tensor.matmul(
                                ps[nt],
                                wt[:, 2 * dp:2 * dp + 2, (fc % 4) * P:(fc % 4 + 1) * P],
                                h1[:, 2 * dp:2 * dp + 2, nt * 512:(nt + 1) * 512],
                                start=(dp == 0), stop=(dp == 3), perf_mode=DR)
                        # evict: (psum/128 + bias) -> bf16 on Act (idle here)
                        nc.scalar.activation(tgt[:, ch, nt * 512:(nt + 1) * 512],
                                             ps[nt], AF.Identity, scale=ISV,
                                             bias=bqkv_c[:, fc:fc + 1])
                    # run the previous chunk's norm-finisher here so its Act
                    # Sqrt never heads the queue before its deps are ready
                    if pending is not None:
                        pending()
                    # sum of squares per head
                    sq = sqq.tile([P, T], BF16, name="sq")
                    nc.vector.tensor_tensor(sq, tgt[:, ch, :], tgt[:, ch, :], OP.mult)
                    if fc < 8:
                        tiles = []
                        for hfq in range(2):
                            prh = psR.tile([1, T], F32, name=f"prh{hfq}")
                            for nt in range(2):
                                nc.tensor.matmul(
                                    prh[:, nt * 512:(nt + 1) * 512],
                                    onesh[:, hfq:hfq + 1],
                                    sq[:, nt * 512:(nt + 1) * 512],
                                    start=True, stop=True)
                            tiles.append(prh)
                        pending = (lambda c=ch, t=tiles: finish_q(c, t))
                    else:
                        tiles = []
                        for j in range(2):
                            pkn = psR.tile([P, NCH], F32, name=f"pkn{j}")
                            for kt in range(NCH):
                                nc.tensor.matmul(
                                    pkn[:, kt:kt + 1],
                                    sq[64 * j:64 * (j + 1), kt * P:(kt + 1) * P],
                                    ones1[64 * j:64 * (j + 1), :],
                                    start=True, stop=True)
                            tiles.append(pkn)
                        pending = (lambda f=fc, t=tiles: finish_k(f, t))
                pending()

            # ------------ attention ------------
            oTp = att.enter_context(tc.tile_pool(name="oTp", bufs=1, side="right"))
            oT = oTp.tile([P, NCH, T], FP8)
            with tc.tile_pool(name="esp", bufs=2) as esp, \
                 tc.tile_pool(name="psS", bufs=2, space="PSUM") as psS, \
                 tc.tile_pool(name="psO", bufs=3, space="PSUM") as psO, \
                 tc.tile_pool(name="onp", bufs=4) as onp:
                for h in range(H):
                    hc, hf = h // 2, h % 2
                    rq = slice(64 * hf, 64 * (hf + 1))
                    es_h = esp.tile([P, NCH, T], FP8, name="es")
                    for ktc in range(NCH):
                        psc = psS.tile([P, T], F32, name="psc")
                        for qt in range(2):
                            nc.tensor.matmul(psc[:, qt * 512:(qt + 1) * 512],
                                             k_t[rq, hc, ktc * P:(ktc + 1) * P],
                                             q_t[rq, hc, qt * 512:(qt + 1) * 512],
                                             start=True, stop=True)
                        nc.scalar.activation(es_h[:, ktc, :], psc, AF.Exp,
                                             bias=eln32_c, scale=rkc[:, h, ktc:ktc + 1])
                    for qt in range(2):
                        qs = slice(qt * 512, (qt + 1) * 512)
                        po = psO.tile([P, 512], F32, name="po")
                        rs = onp.tile([P, 512], F32, name="rs")
                        rsb = onp.tile([P, 512], F32, name="rsb")
                        for kp in range(4):
                            nc.tensor.matmul(
                                po, vx[:, 2 * kp:2 * kp + 2, h, :],
                                es_h[:, 2 * kp:2 * kp + 2, qs],
                                start=(kp == 0), stop=(kp == 3), perf_mode=DR)
                        if hf == 0:
                            # denom at row 64: recip there, DMA row to
                            # partition 0, broadcast full, use rows 0:64
                            nc.vector.reciprocal(rs[64:65, :], po[64:65, :])
                            rse = onp.tile([1, 512], F32, name="rse")
                            nc.sync.dma_start(out=rse, in_=rs[64:65, :])
                            nc.gpsimd.partition_broadcast(rsb, rse)
                            nc.vector.tensor_tensor(oT[0:64, hc, qs], po[0:64, :],
                                                    rsb[0:64, :], OP.mult)
                        else:
                            # denom at row 0: broadcast full, use rows 64:128
                            nc.vector.reciprocal(rs[0:1, :], po[0:1, :])
                            nc.gpsimd.partition_broadcast(rsb, rs[0:1, :])
                            nc.vector.tensor_tensor(oT[64:128, hc, qs], po[64:128, :],
                                                    rsb[64:128, :], OP.mult)

            # ------------ proj + residual ------------
            with tc.tile_pool(name="wpp", bufs=1) as wpp, \
                 tc.tile_pool(name="psP", bufs=3, space="PSUM") as psP:
                wpj = wpp.tile([P, NCH, D], FP8, name="wpj")
                nc.sync.dma_start(out=wpj, in_=wload_ap(wproj, NCH, D, 0))
                nc.sync.dma_start(out=w1a, in_=wload_ap(wfc1, NCH, DM // 2, 0))
                for nt in range(2):
                    for fc in range(8):
                        pp = psP.tile([P, 512], F32, name="pp")
                        for dp in range(4):
                            nc.tensor.matmul(
                                pp, wpj[:, 2 * dp:2 * dp + 2, fc * P:(fc + 1) * P],
                                oT[:, 2 * dp:2 * dp + 2, nt * 512:(nt + 1) * 512],
                                start=(dp == 0), stop=(dp == 3), perf_mode=DR)
                        nc.vector.affine_then_add(
                            x_res[:, fc, nt * 512:(nt + 1) * 512], pp,
                            x_res[:, fc, nt * 512:(nt + 1) * 512],
                            scale=gbs_proj[:, fc:fc + 1],
                            bias=gb_proj[:, fc:fc + 1])

            att.close()  # free h1, q/k, vx, oT, rkc

            # ------------ norm2 + modulate + MLP (single pass, fp8) ------------
            with tc.tile_pool(name="h2p", bufs=1) as h2p, \
                 tc.tile_pool(name="gactp", bufs=1, side="right") as gactp:
                h2 = h2p.tile([P, NCH, T], FP8)
                norm_modulate(scl_n2, 24, 32, h2)
                gact = gactp.tile([P, MCH, T], FP8)
                w1b = gactp.tile([P, NCH, DM // 2], FP8, name="w1b")
                nc.sync.dma_start(out=w1b, in_=wload_ap(wfc1, NCH, DM // 2, DM // 2))
                w2 = gactp.tile([P, MCH, D], FP8, name="w2full")
                nc.sync.dma_start(out=w2, in_=wload_ap(wfc2, MCH, D, 0))
                # nt-outer: fc2 on token-half 0 overlaps fc1/gelu on half 1
                with tc.tile_pool(name="psM", bufs=3, space="PSUM") as psM, \
                     tc.tile_pool(name="psM2", bufs=3, space="PSUM") as psM2:
                    for nt in range(2):
                        ns_ = slice(nt * 512, (nt + 1) * 512)
                        for m in range(MCH):
                            psm = psM.tile([P, 512], F32, name="psm")
                            w1h = w1a if m < 16 else w1b
                            mo = m if m < 16 else m - 16
                            for dp in range(4):
                                nc.tensor.matmul(
                                    psm,
                                    w1h[:, 2 * dp:2 * dp + 2, mo * P:(mo + 1) * P],
                                    h2[:, 2 * dp:2 * dp + 2, ns_],
                                    start=(dp == 0), stop=(dp == 3), perf_mode=DR)
                            nc.scalar.activation(gact[:, m, ns_], psm,
                                                 AF.Gelu_apprx_tanh, scale=ISV,
                                                 bias=bfc1_c[:, m:m + 1])
                        for fc in range(8):
                            ps2 = psM2.tile([P, 512], F32, name="ps2")
                            for dp in range(16):
                                nc.tensor.matmul(
                                    ps2,
                                    w2[:, 2 * dp:2 * dp + 2, fc * P:(fc + 1) * P],
                                    gact[:, 2 * dp:2 * dp + 2, ns_],
                                    start=(dp == 0), stop=(dp == 15), perf_mode=DR)
                            nc.vector.affine_then_add(
                                x_res[:, fc, ns_], ps2, x_res[:, fc, ns_],
                                scale=gbs_fc2[:, fc:fc + 1],
                                bias=gb_fc2[:, fc:fc + 1])
                            if nt == 1:
                                nc.sync.dma_start(out=out[fc * P:(fc + 1) * P, :],
                                                  in_=x_res[:, fc, :])
    nc.compile()
    return nc


_CACHE = {}


def _runner(nc, n_cores=8):
    import jax
    import numpy as _np
    from jax.sharding import Mesh, PartitionSpec, NamedSharding
    from jax.experimental.shard_map import shard_map
    from concourse.bass2jax import _bass_exec_p, install_neuronx_cc_hook, partition_id_tensor

    install_neuronx_cc_hook()
    in_names, out_names, out_avals = [], [], []
    partition_name = nc.partition_id_tensor.name if nc.partition_id_tensor else None
    for alloc in nc.m.functions[0].allocations:
        if not isinstance(alloc, mybir.MemoryLocationSet):
            continue
        nm = alloc.memorylocations[0].name
        if alloc.kind == "ExternalInput":
            if nm != partition_name:
                in_names.append(nm)
        elif alloc.kind == "ExternalOutput":
            out_names.append(nm)
            out_avals.append(jax.core.ShapedArray(tuple(alloc.tensor_shape),
                                                  mybir.dt.np(alloc.dtype)))

    def _body(*args):
        operands = list(args)
        if partition_name is not None:
            operands.append(partition_id_tensor())
        outs = _bass_exec_p.bind(
            *operands,
            out_avals=tuple(out_avals),
            in_names=tuple(in_names + [partition_name] if partition_name else in_names),
            out_names=tuple(out_names),
            lowering_input_output_aliases=(),
            sim_require_finite=False,
            sim_require_nnan=False,
            nc=nc,
        )
        return tuple(outs)

    devices = jax.devices()[:n_cores]
    mesh = Mesh(_np.asarray(devices), ("core",))
    fn = jax.jit(shard_map(_body, mesh=mesh,
                           in_specs=(PartitionSpec("core"),) * len(in_names),
                           out_specs=(PartitionSpec("core"),) * len(out_names),
                           check_rep=False))

    def run(in_maps):
        concat = [_np.concatenate([_np.asarray(m[n]) for m in in_maps], axis=0)
                  for n in in_names]
        args = [jax.device_put(c, NamedSharding(mesh, PartitionSpec("core")))
                for c in concat]
        outs = fn(*args)
        jax.block_until_ready(outs)
        res = []
        for c in range(n_cores):
            d = {}
            for i, nm in enumerate(out_names):
                full = _np.asarray(outs[i])
                d[nm] = full.reshape(n_cores, *out_avals[i].shape)[c]
            res.append(d)
        return res

    return run


def kernel(**inputs):
    """Full (unsharded) inputs -> full (B, T, D) float32 output."""
    if "nc" not in _CACHE:
        _CACHE["nc"] = build_dit(n_cores=8)
        _CACHE["run"] = _runner(_CACHE["nc"], 8)
    in_maps = host_prep(**inputs)
    results = _CACHE["run"](in_maps)
    return host_post(results)


# revision 50
# speedup vs baseline: 1.0274x; 1.0024x over previous
"""nn_DiTBlock on 8 TRN2 NeuronCores: data-parallel over batch (B=8), one
batch element per core. Self-contained: builds the Bass/Tile kernel, shards
inputs on the host (transpose/pack/cast only), runs SPMD via bass2jax/PJRT,
gathers and un-transposes the output.

v2 design: fp8e4(e4m3)+DoubleRow matmuls for qkv/v/attn@v/proj/fc1/fc2
(weights host-prescaled x128, descale folded into evictions), bf16 for adaLN
and attention scores, f32 residual + PSUM. exp scaled by 1/32 (cancels in
softmax); k-rmsnorm folded into the exp's per-partition scale; odd heads'
attn@v written directly to PSUM partitions 64:128 (no partition-move DMAs);
single-pass MLP; multi-chunk batched weight DMAs."""

import numpy as np
from contextlib import ExitStack

import concourse.bass as bass
import concourse.mybir as mybir
import concourse.tile as tile
from concourse import bacc


F32 = mybir.dt.float32
F32R = mybir.dt.float32r
BF16 = mybir.dt.bfloat16
FP8 = mybir.dt.float8e4
AF = mybir.ActivationFunctionType
OP = mybir.AluOpType
DR = mybir.MatmulPerfMode.DoubleRow

B, T, D, H = 8, 1024, 1024, 16
HD = D // H          # 64
DM = 4 * D           # 4096
NCH = D // 128       # 8
MCH = DM // 128      # 32
P = 128
WS = 128.0           # fp8 weight pre-scale (host)
ISV = 1.0 / WS
ELN32 = -3.4657359027997265  # -ln(32): exp pre-scale so fp8 es stays < 240


def host_prep(x, c, g1, g2, gq, gk, Wqkv, bqkv, Wproj, bproj,
              Wfc1, bfc1, Wfc2, bfc2, Wada, bada):
    import ml_dtypes
    E4 = mybir.dt.np(FP8)

    def packT(W, npdt, scale=1.0):  # (F, K) -> (K//128, 128, F) contiguous
        Wt = np.ascontiguousarray(np.asarray(W, np.float32).T * scale).astype(npdt)
        K, F = Wt.shape
        return np.ascontiguousarray(Wt.reshape(K // 128, 128, F))

    f32 = np.float32
    com = {
        "wqkv": packT(Wqkv, E4, WS), "wproj": packT(Wproj, E4, WS),
        "wfc1": packT(Wfc1, E4, WS), "wfc2": packT(Wfc2, E4, WS),
        "wada8": packT(np.asarray(Wada, np.float32)[
            np.r_[0:2048, 3072:5120], :], E4, WS),
        "wadab": packT(np.asarray(Wada, np.float32)[
            np.r_[2048:3072, 5120:6144], :], ml_dtypes.bfloat16),
        "bqkv": np.asarray(bqkv, f32), "bproj": np.asarray(bproj, f32),
        "bfc1": np.asarray(bfc1, f32), "bfc2": np.asarray(bfc2, f32),
        "bada": np.asarray(bada, f32),
        "g": np.stack([np.asarray(g1)[0], np.asarray(g2)[0],
                       np.asarray(gq)[0], np.asarray(gk)[0]]).astype(f32),
    }
    in_maps = []
    for b in range(B):
        m = dict(com)
        m["xt"] = np.ascontiguousarray(np.asarray(x[b], f32).T)
        m["cvec"] = np.asarray(c[b], f32)
        in_maps.append(m)
    return in_maps


def host_post(results):
    return np.ascontiguousarray(
        np.stack([r["out"].T for r in results]).astype(np.float32))


def col_ap(handle, nch):
    """DRAM (nch*128,) viewed as [128, nch]: tile[p, ch] = v[ch*128+p]."""
    return bass.AP(tensor=handle, offset=0, ap=[[1, P], [P, nch]])


def bc_ap(handle, n, offset=0):
    """DRAM (n,) broadcast-read to [128, n] (partition stride 0)."""
    return bass.AP(tensor=handle, offset=offset, ap=[[0, P], [1, n]])


def wload_ap(handle, kch, cols, col0):
    """DRAM weight pack [KCH,128,F] -> [128, kch, cols] AP at col offset."""
    F = handle.shape[2]
    return bass.AP(tensor=handle, offset=col0,
                   ap=[[F, P], [P * F, kch], [1, cols]])


def build_dit(n_cores=8):
    nc = bacc.Bacc("TRN2", target_bir_lowering=False, debug=False,
                   num_devices=n_cores)

    xt = nc.dram_tensor("xt", [D, T], F32, kind="ExternalInput")
    cin = nc.dram_tensor("cvec", [D], F32, kind="ExternalInput")
    g = nc.dram_tensor("g", [4], F32, kind="ExternalInput")
    wqkv = nc.dram_tensor("wqkv", [NCH, P, 3 * D], FP8, kind="ExternalInput")
    wproj = nc.dram_tensor("wproj", [NCH, P, D], FP8, kind="ExternalInput")
    wfc1 = nc.dram_tensor("wfc1", [NCH, P, DM], FP8, kind="ExternalInput")
    wfc2 = nc.dram_tensor("wfc2", [MCH, P, D], FP8, kind="ExternalInput")
    wada8 = nc.dram_tensor("wada8", [NCH, P, 4 * D], FP8, kind="ExternalInput")
    wadab = nc.dram_tensor("wadab", [NCH, P, 2 * D], BF16, kind="ExternalInput")
    bqkv = nc.dram_tensor("bqkv", [3 * D], F32, kind="ExternalInput")
    bproj = nc.dram_tensor("bproj", [D], F32, kind="ExternalInput")
    bfc1 = nc.dram_tensor("bfc1", [DM], F32, kind="ExternalInput")
    bfc2 = nc.dram_tensor("bfc2", [D], F32, kind="ExternalInput")
    bada = nc.dram_tensor("bada", [6 * D], F32, kind="ExternalInput")
    out = nc.dram_tensor("out", [D, T], F32, kind="ExternalOutput")

    with tile.TileContext(nc, pool_alloc_mode="queue") as tc:
        with ExitStack() as X:
            const = X.enter_context(tc.tile_pool(name="const", bufs=1))
            resid = X.enter_context(tc.tile_pool(name="resid", bufs=1))
            dram = X.enter_context(tc.tile_pool(name="dram", bufs=1, space="DRAM"))

            # ---------------- constants ----------------
            g_bc = const.tile([P, 4], F32)
            nc.sync.dma_start(out=g_bc, in_=bc_ap(g, 4))
            gsq = const.tile([P, 4], F32)
            nc.vector.tensor_tensor(gsq, g_bc, g_bc, OP.mult)
            ginv2 = const.tile([P, 4], F32)
            nc.vector.reciprocal(ginv2, gsq)
            # Rsqrt scales: rinv = rsqrt(ps * scl)
            scl_n1 = const.tile([P, 1], F32)
            nc.vector.tensor_scalar_mul(scl_n1, ginv2[:, 0:1], 1.0 / D)
            scl_n2 = const.tile([P, 1], F32)
            nc.vector.tensor_scalar_mul(scl_n2, ginv2[:, 1:2], 1.0 / D)
            scl_q = const.tile([P, 1], F32)
            nc.vector.tensor_copy(scl_q, ginv2[:, 2:3])
            scl_k = const.tile([P, 1], F32)
            nc.vector.tensor_scalar_mul(scl_k, ginv2[:, 3:4], 1.0 / HD)

            ones1_f = const.tile([P, 1], F32)
            nc.gpsimd.memset(ones1_f, 1.0)
            ones1 = const.tile([P, 1], BF16)
            nc.vector.tensor_copy(ones1, ones1_f)
            onesh_f = const.tile([P, 2], F32)
            nc.gpsimd.memset(onesh_f, 0.0)
            nc.gpsimd.memset(onesh_f[0:64, 0:1], 1.0)
            nc.gpsimd.memset(onesh_f[64:128, 1:2], 1.0)
            onesh = const.tile([P, 2], BF16)
            nc.vector.tensor_copy(onesh, onesh_f)

            bqkv_c = const.tile([P, 3 * D // P], F32)
            nc.sync.dma_start(out=bqkv_c, in_=col_ap(bqkv, 3 * D // P))
            bproj_c = const.tile([P, NCH], F32)
            nc.sync.dma_start(out=bproj_c, in_=col_ap(bproj, NCH))
            bfc1_c = const.tile([P, MCH], F32)
            nc.sync.dma_start(out=bfc1_c, in_=col_ap(bfc1, MCH))
            bfc2_c = const.tile([P, NCH], F32)
            nc.sync.dma_start(out=bfc2_c, in_=col_ap(bfc2, NCH))
            vbias_bc = const.tile([P, D], F32)
            nc.sync.dma_start(out=vbias_bc, in_=bc_ap(bqkv, D, offset=2 * D))
            eln32_c = const.tile([P, 1], F32)
            nc.gpsimd.memset(eln32_c, ELN32)

            x_res = resid.tile([P, NCH, T], F32)
            for j in range(4):
                nc.sync.dma_start(
                    out=x_res[:, 2 * j:2 * j + 2, :],
                    in_=bass.AP(tensor=xt, offset=2 * j * P * T,
                                ap=[[T, P], [P * T, 2], [1, T]]))

            c_pm = const.tile([P, NCH], F32)
            nc.sync.dma_start(out=c_pm, in_=col_ap(cin, NCH))
            cs_pm = const.tile([P, NCH], BF16)
            nc.scalar.activation(cs_pm, c_pm, AF.Silu)

            # ---------------- adaLN (bf16) ----------------
            ada_scr = dram.tile([1, 6 * D], F32)
            ada_sb = const.tile([1, 6 * D], F32)
            cs8 = const.tile([P, NCH], FP8)
            nc.vector.tensor_copy(cs8, cs_pm)
            with tc.tile_pool(name="wadap", bufs=3) as wp, \
                 tc.tile_pool(name="psA", bufs=2, space="PSUM") as psA:
                # shift/scale blocks in fp8 (x128), original col offsets:
                # wada8 = [sh_msa|sc_msa|sh_mlp|sc_mlp], wadab = [g_msa|g_mlp]
                for nb in range(8):
                    og = nb * 512 if nb < 4 else 3072 + (nb - 4) * 512
                    wt8 = wp.tile([P, NCH, 512], FP8, name="wt8")
                    nc.sync.dma_start(out=wt8, in_=wload_ap(wada8, NCH, 512, nb * 512))
                    pa = psA.tile([1, 512], F32, name="pa")
                    for d in range(NCH):
                        nc.tensor.matmul(pa, cs8[:, d:d + 1], wt8[:, d, :],
                                         start=(d == 0), stop=(d == NCH - 1))
                    nc.vector.tensor_scalar_mul(ada_sb[:, og:og + 512], pa, ISV)
                for nb in range(8):
                    og = (2048 + nb * 256) if nb < 4 else (5120 + (nb - 4) * 256)
                    wt = wp.tile([P, NCH, 256], BF16, name="wt")
                    nc.sync.dma_start(out=wt, in_=wload_ap(wadab, NCH, 256, nb * 256))
                    pa = psA.tile([1, 256], F32, name="pab")
                    for d in range(NCH):
                        nc.tensor.matmul(pa, cs_pm[:, d:d + 1], wt[:, d, :],
                                         start=(d == 0), stop=(d == NCH - 1))
                    nc.vector.tensor_copy(ada_sb[:, og:og + 256], pa)
            nc.sync.dma_start(out=ada_scr, in_=ada_sb)
            adaT = const.tile([P, 48], F32)
            nc.sync.dma_start(out=adaT, in_=bass.AP(tensor=ada_scr.tensor, offset=0,
                                                    ap=[[1, P], [P, 48]]))
            badaT = const.tile([P, 48], F32)
            nc.sync.dma_start(out=badaT, in_=col_ap(bada, 48))
            nc.vector.tensor_tensor(adaT, adaT, badaT, OP.add)
            # cols: shift_msa 0:8 | scale_msa 8:16 | gate_msa 16:24
            #       shift_mlp 24:32 | scale_mlp 32:40 | gate_mlp 40:48
            nc.vector.tensor_scalar_add(adaT[:, 8:16], adaT[:, 8:16], 1.0)
            nc.vector.tensor_scalar_add(adaT[:, 32:40], adaT[:, 32:40], 1.0)
            gb_proj = const.tile([P, NCH], F32)
            nc.vector.tensor_tensor(gb_proj, adaT[:, 16:24], bproj_c, OP.mult)
            gbs_proj = const.tile([P, NCH], F32)
            nc.vector.tensor_scalar_mul(gbs_proj, adaT[:, 16:24], ISV)
            gb_fc2 = const.tile([P, NCH], F32)
            nc.vector.tensor_tensor(gb_fc2, adaT[:, 40:48], bfc2_c, OP.mult)
            gbs_fc2 = const.tile([P, NCH], F32)
            nc.vector.tensor_scalar_mul(gbs_fc2, adaT[:, 40:48], ISV)

            def norm_modulate(scl, sh_col, sc_col, h_out):
                """x_res (f32) -> h_out (fp8): rmsnorm + adaLN modulate.
                Token-halved so the consumer can start on half 0 while the
                producer of x_res is still finishing half 1."""
                with tc.tile_pool(name="sqp", bufs=3) as sqp, \
                     tc.tile_pool(name="psN", bufs=1, space="PSUM") as psN, \
                     tc.tile_pool(name="nrm", bufs=2) as nrm, \
                     tc.tile_pool(name="xnp", bufs=3) as xnp:
                    pss = psN.tile([1, T], F32, name="pss")
                    for t2 in range(2):
                        ts_ = slice(t2 * 512, (t2 + 1) * 512)
                        for j in range(NCH):
                            xsq = sqp.tile([P, 512], BF16, name="xsq")
                            nc.scalar.activation(xsq, x_res[:, j, ts_], AF.Square)
                            nc.tensor.matmul(pss[:, ts_], ones1, xsq,
                                             start=(j == 0), stop=(j == NCH - 1))
                        rr = nrm.tile([1, 512], F32, name="rr")
                        nc.scalar.activation(rr, pss[:, ts_], AF.Sqrt,
                                             scale=scl[0:1, :])
                        rinv = nrm.tile([1, 512], F32, name="rinv")
                        nc.vector.reciprocal(rinv, rr)
                        rbc = nrm.tile([P, 512], F32, name="rbc")
                        nc.gpsimd.partition_broadcast(rbc, rinv)
                        for j in range(NCH):
                            xn = xnp.tile([P, 512], F32, name="xn")
                            nc.vector.tensor_tensor(xn, x_res[:, j, ts_], rbc,
                                                    OP.mult)
                            nc.gpsimd.tensor_scalar(h_out[:, j, ts_], xn,
                                                    adaT[:, sc_col + j:sc_col + j + 1],
                                                    adaT[:, sh_col + j:sh_col + j + 1],
                                                    OP.mult, OP.add)

            # fc1 weights tile created before the attention pools (so they
            # can close first); its load is emitted at proj time, landing
            # during attention when the wire is idle
            mlpw = X.enter_context(tc.tile_pool(name="mlpw", bufs=1))
            w1a = mlpw.tile([P, NCH, DM // 2], FP8)

            att = ExitStack()
            h1p = att.enter_context(tc.tile_pool(name="h1p", bufs=1, side="right"))
            h1 = h1p.tile([P, NCH, T], FP8)
            # ------------ norm1 + modulate ------------
            norm_modulate(scl_n1, 0, 8, h1)

            # ------------ q, k (feature-major bf16) + per-head rmsnorm ------------
            qp_ = att.enter_context(tc.tile_pool(name="qp_", bufs=1))
            kp_ = att.enter_context(tc.tile_pool(name="kp_", bufs=1))
            q_t = qp_.tile([P, NCH, T], BF16)
            k_t = kp_.tile([P, NCH, T], BF16)
            rkcp = att.enter_context(tc.tile_pool(name="rkcp", bufs=1))
            rkc = rkcp.tile([P, H, NCH], F32)  # 1/|k| per k-token, head-major

            # ------------ v (token-major fp8, ones-augmented) ------------
            # vx per-head 128-col slot: even h = [v(0:64) | ones@64 | 0],
            # odd h = [0 | ones@63 | v(64:128)]; attn@v DR outputs are then
            # always full [128, N] (walrus requires that) and odd heads land
            # on PSUM partitions 64:128 directly.
            vxp = att.enter_context(tc.tile_pool(name="vxp", bufs=1))
            vx = vxp.tile([P, NCH, H, P], FP8)   # [ktok][ktc][head][col]
            nc.gpsimd.memset(vx, 0.0)
            for h in range(H):
                oc = HD if h % 2 == 0 else 0
                nc.gpsimd.memset(vx[:, :, h, oc:oc + 1], 1.0)
            with tc.tile_pool(name="wvp", bufs=2) as wvp, \
                 tc.tile_pool(name="psV", bufs=3, space="PSUM") as psV:
                for nq in range(2):
                    wv = wvp.tile([P, NCH, 512], FP8, name="wv")
                    nc.sync.dma_start(out=wv,
                                      in_=wload_ap(wqkv, NCH, 512, 2 * D + nq * 512))
                    for t8 in range(NCH):
                        pv = psV.tile([P, 512], F32, name="pv")
                        for dp in range(4):
                            nc.tensor.matmul(
                                pv, h1[:, 2 * dp:2 * dp + 2, t8 * P:(t8 + 1) * P],
                                wv[:, 2 * dp:2 * dp + 2, :],
                                start=(dp == 0), stop=(dp == 3), perf_mode=DR)
                        # heads alternate col-base 0 (even) / 64 (odd) in vx
                        vblk = vx[:, t8, :, :].rearrange(
                            "p h c -> p (h c)").rearrange(
                            "p (i r) -> p i r", r=256)  # [P, 8, 256]
                        for par in range(2):
                            nc.vector.scalar_tensor_tensor(
                                vblk[:, 4 * nq:4 * nq + 4,
                                     192 * par:192 * par + HD],
                                pv.rearrange("p (i r) -> p i r", r=128)[
                                    :, :, par * HD:(par + 1) * HD], ISV,
                                vbias_bc[:, nq * 512:(nq + 1) * 512].rearrange(
                                    "p (i r) -> p i r", r=128)[
                                    :, :, par * HD:(par + 1) * HD],
                                OP.mult, OP.add)

            with tc.tile_pool(name="wqp", bufs=2) as wqp, \
                 tc.tile_pool(name="sqq", bufs=2) as sqq, \
                 tc.tile_pool(name="psD", bufs=2, space="PSUM") as psD, \
                 tc.tile_pool(name="psR", bufs=1, space="PSUM") as psR, \
                 tc.tile_pool(name="nrq", bufs=1) as nrq:
                def finish_q(ch, tiles):
                    for hfq in range(2):
                        prh = tiles[hfq]
                        rr2 = nrq.tile([1, T], BF16, name=f"rr2{hfq}")
                        nc.scalar.activation(rr2, prh, AF.Sqrt,
                                             scale=scl_q[0:1, :])
                        rinv_sb = nrq.tile([1, T], BF16, name=f"ri{hfq}")
                        with nc.allow_low_precision(reason="1/|q| bf16"):
                            nc.vector.reciprocal(rinv_sb, rr2)
                        rbcq = nrq.tile([P, T], BF16, name=f"rbcq{hfq}")
                        nc.gpsimd.partition_broadcast(rbcq, rinv_sb)
                        hs = slice(64 * hfq, 64 * (hfq + 1))
                        nc.vector.tensor_tensor(q_t[hs, ch, :], q_t[hs, ch, :],
                                                rbcq[hs, :], OP.mult)

                def finish_k(fc, tiles):
                    for j in range(2):
                        hidx = 2 * (fc - 8) + j
                        rrk = nrq.tile([P, NCH], F32, name="rrk")
                        nc.scalar.activation(rrk, tiles[j], AF.Sqrt, scale=scl_k)
                        nc.vector.reciprocal(rkc[:, hidx, :], rrk)

                pending = None
                for fc in range(16):  # q: 0..7, k: 8..15
                    if fc % 4 == 0:
                        wt = wqp.tile([P, NCH, 512], FP8, name="wt")
                        nc.sync.dma_start(out=wt, in_=wload_ap(wqkv, NCH, 512, fc * P))
                    tgt = q_t if fc < 8 else k_t
                    ch = fc % 8
                    ps = [psD.tile([P, 512], F32, name="ps") for _ in range(2)]
                    for nt in range(2):
                        for dp in range(4):
                            nc.tensor.matmul(
                                ps[nt],
                                wt[:, 2 * dp:2 * dp + 2, (fc % 4) * P:(fc % 4 + 1) * P],
                                h1[:, 2 * dp:2 * dp + 2, nt * 512:(nt + 1) * 512],
                                start=(dp == 0), stop=(dp == 3), perf_mode=DR)
                        # evict: (psum/128 + bias) -> bf16 on Act (idle here)
                        nc.scalar.activation(tgt[:, ch, nt * 512:(nt + 1) * 512],
                                             ps[nt], AF.Identity, scale=ISV,
                                             bias=bqkv_c[:, fc:fc + 1])
                    # run the previous chunk's norm-finisher here so its Act
                    # Sqrt never heads the queue before its deps are ready
                    if pending is not None:
                        pending()
                    # sum of squares per head
                    sq = sqq.tile([P, T], BF16, name="sq")
                    nc.vector.tensor_tensor(sq, tgt[:, ch, :], tgt[:, ch, :], OP.mult)
                    if fc < 8:
                        tiles = []
                        for hfq in range(2):
                            prh = psR.tile([1, T], F32, name=f"prh{hfq}")
                            for nt in range(2):
                                nc.tensor.matmul(
                                    prh[:, nt * 512:(nt + 1) * 512],
                                    onesh[:, hfq:hfq + 1],
                                    sq[:, nt * 512:(nt + 1) * 512],
                                    start=True, stop=True)
                            tiles.append(prh)
                        pending = (lambda c=ch, t=tiles: finish_q(c, t))
                    else:
                        tiles = []
                        for j in range(2):
                            pkn = psR.tile([P, NCH], F32, name=f"pkn{j}")
                            for kt in range(NCH):
                                nc.tensor.matmul(
                                    pkn[:, kt:kt + 1],
                                    sq[64 * j:64 * (j + 1), kt * P:(kt + 1) * P],
                                    ones1[64 * j:64 * (j + 1), :],
                                    start=True, stop=True)
                            tiles.append(pkn)
                        pending = (lambda f=fc, t=tiles: finish_k(f, t))
                pending()

            # ------------ attention ------------
            oTp = att.enter_context(tc.tile_pool(name="oTp", bufs=1, side="right"))
            oT = oTp.tile([P, NCH, T], FP8)
            with tc.tile_pool(name="esp", bufs=2) as esp, \
                 tc.tile_pool(name="psS", bufs=2, space="PSUM") as psS, \
                 tc.tile_pool(name="psO", bufs=3, space="PSUM") as psO, \
                 tc.tile_pool(name="onp", bufs=4) as onp:
                for h in range(H):
                    hc, hf = h // 2, h % 2
                    rq = slice(64 * hf, 64 * (hf + 1))
                    es_h = esp.tile([P, NCH, T], FP8, name="es")
                    for ktc in range(NCH):
                        psc = psS.tile([P, T], F32, name="psc")
                        for qt in range(2):
                            nc.tensor.matmul(psc[:, qt * 512:(qt + 1) * 512],
                                             k_t[rq, hc, ktc * P:(ktc + 1) * P],
                                             q_t[rq, hc, qt * 512:(qt + 1) * 512],
                                             start=True, stop=True)
                        nc.scalar.activation(es_h[:, ktc, :], psc, AF.Exp,
                                             bias=eln32_c, scale=rkc[:, h, ktc:ktc + 1])
                    for qt in range(2):
                        qs = slice(qt * 512, (qt + 1) * 512)
                        po = psO.tile([P, 512], F32, name="po")
                        rs = onp.tile([P, 512], F32, name="rs")
                        rsb = onp.tile([P, 512], F32, name="rsb")
                        for kp in range(4):
                            nc.tensor.matmul(
                                po, vx[:, 2 * kp:2 * kp + 2, h, :],
                                es_h[:, 2 * kp:2 * kp + 2, qs],
                                start=(kp == 0), stop=(kp == 3), perf_mode=DR)
                        if hf == 0:
                            # denom at row 64: recip there, DMA row to
                            # partition 0, broadcast full, use rows 0:64
                            nc.vector.reciprocal(rs[64:65, :], po[64:65, :])
                            rse = onp.tile([1, 512], F32, name="rse")
                            nc.sync.dma_start(out=rse, in_=rs[64:65, :])
                            nc.gpsimd.partition_broadcast(rsb, rse)
                            nc.vector.tensor_tensor(oT[0:64, hc, qs], po[0:64, :],
                                                    rsb[0:64, :], OP.mult)
                        else:
                            # denom at row 0: broadcast full, use rows 64:128
                            nc.vector.reciprocal(rs[0:1, :], po[0:1, :])
                            nc.gpsimd.partition_broadcast(rsb, rs[0:1, :])
                            nc.vector.tensor_tensor(oT[64:128, hc, qs], po[64:128, :],
                                                    rsb[64:128, :], OP.mult)

            # ------------ proj + residual ------------
            with tc.tile_pool(name="wpp", bufs=1) as wpp, \
                 tc.tile_pool(name="psP", bufs=4, space="PSUM") as psP:
                wpj = wpp.tile([P, NCH, D], FP8, name="wpj")
                nc.sync.dma_start(out=wpj, in_=wload_ap(wproj, NCH, D, 0))
                nc.sync.dma_start(out=w1a, in_=wload_ap(wfc1, NCH, DM // 2, 0))
                for nt in range(2):
                    for fc in range(8):
                        pp = psP.tile([P, 512], F32, name="pp")
                        for dp in range(4):
                            nc.tensor.matmul(
                                pp, wpj[:, 2 * dp:2 * dp + 2, fc * P:(fc + 1) * P],
                                oT[:, 2 * dp:2 * dp + 2, nt * 512:(nt + 1) * 512],
                                start=(dp == 0), stop=(dp == 3), perf_mode=DR)
                        nc.vector.affine_then_add(
                            x_res[:, fc, nt * 512:(nt + 1) * 512], pp,
                            x_res[:, fc, nt * 512:(nt + 1) * 512],
                            scale=gbs_proj[:, fc:fc + 1],
                            bias=gb_proj[:, fc:fc + 1])

            att.close()  # free h1, q/k, vx, oT, rkc

            # ------------ norm2 + modulate + MLP (single pass, fp8) ------------
            with tc.tile_pool(name="h2p", bufs=1) as h2p, \
                 tc.tile_pool(name="gactp", bufs=1, side="right") as gactp:
                h2 = h2p.tile([P, NCH, T], FP8)
                norm_modulate(scl_n2, 24, 32, h2)
                gact = gactp.tile([P, MCH, T], FP8)
                w1b = gactp.tile([P, NCH, DM // 2], FP8, name="w1b")
                nc.sync.dma_start(out=w1b, in_=wload_ap(wfc1, NCH, DM // 2, DM // 2))
                w2 = gactp.tile([P, MCH, D], FP8, name="w2full")
                nc.sync.dma_start(out=w2, in_=wload_ap(wfc2, MCH, D, 0))
                # nt-outer: fc2 on token-half 0 overlaps fc1/gelu on half 1
                with tc.tile_pool(name="psM", bufs=4, space="PSUM") as psM, \
                     tc.tile_pool(name="psM2", bufs=4, space="PSUM") as psM2:
                    for nt in range(2):
                        ns_ = slice(nt * 512, (nt + 1) * 512)
                        for m in range(MCH):
                            psm = psM.tile([P, 512], F32, name="psm")
                            w1h = w1a if m < 16 else w1b
                            mo = m if m < 16 else m - 16
                            for dp in range(4):
                                nc.tensor.matmul(
                                    psm,
                                    w1h[:, 2 * dp:2 * dp + 2, mo * P:(mo + 1) * P],
                                    h2[:, 2 * dp:2 * dp + 2, ns_],
                                    start=(dp == 0), stop=(dp == 3), perf_mode=DR)
                            nc.scalar.activation(gact[:, m, ns_], psm,
                                                 AF.Gelu_apprx_tanh, scale=ISV,
                                                 bias=bfc1_c[:, m:m + 1])
                        for fc in range(8):
                            ps2 = psM2.tile([P, 512], F32, name="ps2")
                            for dp in range(16):
                                nc.tensor.matmul(
                                    ps2,
                                    w2[:, 2 * dp:2 * dp + 2, fc * P:(fc + 1) * P],
                                    gact[:, 2 * dp:2 * dp + 2, ns_],
                                    start=(dp == 0), stop=(dp == 15), perf_mode=DR)
                            nc.vector.affine_then_add(
                                x_res[:, fc, ns_], ps2, x_res[:, fc, ns_],
                                scale=gbs_fc2[:, fc:fc + 1],
                                bias=gb_fc2[:, fc:fc + 1])
                            if nt == 1:
                                nc.sync.dma_start(out=out[fc * P:(fc + 1) * P, :],
                                                  in_=x_res[:, fc, :])
    nc.compile()
    return nc


_CACHE = {}


def _runner(nc, n_cores=8):
    import jax
    import numpy as _np
    from jax.sharding import Mesh, PartitionSpec, NamedSharding
    from jax.experimental.shard_map import shard_map
    from concourse.bass2jax import _bass_exec_p, install_neuronx_cc_hook, partition_id_tensor

    install_neuronx_cc_hook()
    in_names, out_names, out_avals = [], [], []
    partition_name = nc.partition_id_tensor.name if nc.partition_id_tensor else None
    for alloc in nc.m.functions[0].allocations:
        if not isinstance(alloc, mybir.MemoryLocationSet):
            continue
        nm = alloc.memorylocations[0].name
        if alloc.kind == "ExternalInput":
            if nm != partition_name:
                in_names.append(nm)
        elif alloc.kind == "ExternalOutput":
            out_names.append(nm)
            out_avals.append(jax.core.ShapedArray(tuple(alloc.tensor_shape),
                                                  mybir.dt.np(alloc.dtype)))

    def _body(*args):
        operands = list(args)
        if partition_name is not None:
            operands.append(partition_id_tensor())
        outs = _bass_exec_p.bind(
            *operands,
            out_avals=tuple(out_avals),
            in_names=tuple(in_names + [partition_name] if partition_name else in_names),
            out_names=tuple(out_names),
            lowering_input_output_aliases=(),
            sim_require_finite=False,
            sim_require_nnan=False,
            nc=nc,
        )
        return tuple(outs)

    devices = jax.devices()[:n_cores]
    mesh = Mesh(_np.asarray(devices), ("core",))
    fn = jax.jit(shard_map(_body, mesh=mesh,
                           in_specs=(PartitionSpec("core"),) * len(in_names),
                           out_specs=(PartitionSpec("core"),) * len(out_names),
                           check_rep=False))

    def run(in_maps):
        concat = [_np.concatenate([_np.asarray(m[n]) for m in in_maps], axis=0)
                  for n in in_names]
        args = [jax.device_put(c, NamedSharding(mesh, PartitionSpec("core")))
                for c in concat]
        outs = fn(*args)
        jax.block_until_ready(outs)
        res = []
        for c in range(n_cores):
            d = {}
            for i, nm in enumerate(out_names):
                full = _np.asarray(outs[i])
                d[nm] = full.reshape(n_cores, *out_avals[i].shape)[c]
            res.append(d)
        return res

    return run


def kernel(**inputs):
    """Full (unsharded) inputs -> full (B, T, D) float32 output."""
    if "nc" not in _CACHE:
        _CACHE["nc"] = build_dit(n_cores=8)
        _CACHE["run"] = _runner(_CACHE["nc"], 8)
    in_maps = host_prep(**inputs)
    results = _CACHE["run"](in_maps)
    return host_post(results)
